# revision 1
# baseline (speedup 1.0000x reference)
"""Trainium2 Bass kernel for nn_Attention_15857019256917 (ViTDet-style attention
with decomposed relative position bias).

Sharding: data-parallel over B (2) x head-parallel (12 heads -> 4 groups of 3)
= 8 cores. Each core computes 3 heads of attention for one batch element plus
its partial output projection (rows of proj_w for its heads); the host sums the
4 partials per batch element (row-parallel linear unshard) and adds the bias
(with the v-bias folded in exactly: P@(V+1 bv^T)/l = PV/l + bv^T).

Device algorithm per core (fp32 matmuls on the logit path; float32r hi+lo
splits — which together carry full fp32 precision — on the bias tables and V;
P^T itself is stored float32r, the only reduced-precision link, ~1e-5 effect):
  qT/kT = (x @ Wqk)^T via out-transposed matmuls (channels on partitions)
  V     = x @ Wv in natural layout, ones-column per head, split into f32r hi/lo
  rel tables rel_wT/rel_hT [48|48, S] by per-row matmuls, split into f32r hi/lo
  S^T tiles [128k, qw] = fp32 K^T q matmul + one-hot bias matmuls (hi+lo)
  P^T = exp(S^T) via ACT -> f32r (no max subtraction: logits bounded, |S|<10)
  out^T|l = (Vhi|1)^T P^T + (Vlo|0)^T P^T   (M=65; row 64 = softmax denoms)
  y += (out_h^T.T @ Wp_h) * (1/l_h) per-partition scaling, summed over heads
"""
import sys

sys.path.insert(0, "/opt/trn_rl_repo")

import numpy as np

import concourse.bass as bass
import concourse.bacc as bacc
import concourse.tile as tile
from concourse import mybir
from concourse.masks import make_identity

F32 = mybir.dt.float32
F32R = mybir.dt.float32r
ACTF = mybir.ActivationFunctionType

B, H, W, D = 2, 48, 48, 768
NH, HD = 12, 64
S = H * W                      # 2304
SCALE = HD ** -0.5
N_CORES = 8
NHC = 3                        # heads per core
KT = S // 128                  # 18 key tiles
TOKT = S // 128                # 18 token tiles
KCH = D // 128                 # 6 contraction chunks
QT = [(0, 512), (512, 512), (1024, 512), (1536, 512), (2048, 256)]
VST = NHC * (HD + 1)           # 195: per-ktile V layout [v_h0|1|v_h1|1|v_h2|1]
WAVES = [(0, 2), (2, 4), (4, 6)]   # xT chunk waves (chunks [lo, hi))


def _ap(t, off_elems, dims):
    """Raw AP on tile t: partition dim copied, free dims = [[step, count], ...]."""
    return bass.AP(tensor=t.tensor, offset=t.offset + off_elems, ap=[t.ap[0]] + dims)


def _emit(tc, nc, aps, pfx="", p_split=True, bias_split=True,
          stop_after="full", dbg=None):
    xT, wqk, bqk, wv, wp, RhT, RwT, Ecomb, zeros16, y = aps
    from contextlib import ExitStack

    with ExitStack() as es:
        consts = es.enter_context(tc.tile_pool(name=pfx + "consts", bufs=1))
        big = es.enter_context(tc.tile_pool(name=pfx + "big", bufs=1))

        RhT_sb = consts.tile([HD, S], F32)
        nc.sync.dma_start(out=RhT_sb, in_=RhT)
        RwT_sb = consts.tile([HD, S], F32)
        nc.sync.dma_start(out=RwT_sb, in_=RwT)
        Ec_sb = consts.tile([112, S], F32R)
        nc.gpsimd.dma_start(out=Ec_sb, in_=Ecomb)

        qT = big.tile([128, NHC * S], F32)
        kT = big.tile([128, NHC * S], F32)
        outT = [big.tile([HD + 1, S], F32, name=f"outT{j}", tag=f"outT{j}")
                for j in range(NHC)]
        reciplc = big.tile([128, NHC * TOKT], F32)
        # V (with interleaved ones columns), f32r hi/lo split when p_split
        if p_split:
            v_hi = big.tile([128, TOKT * VST], F32R, name="v_hi", tag="v_hi")
            v_lo = big.tile([128, TOKT * VST], F32R, name="v_lo", tag="v_lo")
        else:
            v_hi = big.tile([128, TOKT * VST], F32, name="v_hi", tag="v_hi")
            v_lo = None

        # ---------------- phase 1: qkv projections ----------------
        with tc.tile_pool(name=pfx + "ph1", bufs=1) as ph1, \
             tc.tile_pool(name=pfx + "xw", bufs=2) as xw, \
             tc.tile_pool(name=pfx + "ps_qk", bufs=2, space="PSUM") as ps_qk, \
             tc.tile_pool(name=pfx + "ps_v", bufs=2, space="PSUM") as ps_v:
            wqk_sb = ph1.tile([128, KCH * 2 * NHC * HD], F32)   # [128, 6*384]
            wv_sb = ph1.tile([128, KCH * NHC * HD], F32)        # [128, 6*192]
            bqk_sb = ph1.tile([128, NHC], F32)                  # half-stacked biases
            nc.sync.dma_start(out=bqk_sb, in_=bqk)
            v32 = (ph1.tile([128, TOKT * VST], F32, name="v32")
                   if p_split else v_hi)
            nc.vector.memset(_ap(v32, HD, [[VST, TOKT], [HD + 1, NHC]]), 1.0)
            for k in range(KCH):
                nc.sync.dma_start(out=wqk_sb[:, k * 384:(k + 1) * 384],
                                  in_=wqk[k * 128:(k + 1) * 128, :])
                nc.sync.dma_start(out=wv_sb[:, k * 192:(k + 1) * 192],
                                  in_=wv[k * 128:(k + 1) * 128, :])

            # M-tiles (128 rows = two 64-channel halves):
            #   T0=[q0|q1]  T1=[q2|k0]  T2=[k1|k2]
            # low halves copy straight to rows 0-63 of their dest tensor; high
            # halves park in the dest tensor's padding rows 64-127 (same column
            # range), then an intra-tensor DMA partition-shifts them down.
            lo_dest = [(qT, 0), (qT, 2), (kT, 1)]
            hi_dest = [(qT, 1), (kT, 0), (kT, 2)]
            for wave, (klo, khi) in enumerate(WAVES):
                xs = []
                for k in range(klo, khi):
                    xt = xw.tile([128, S], F32, name=f"x{k}", tag="x")
                    nc.sync.dma_start(out=xt, in_=xT[k * 128:(k + 1) * 128, :])
                    xs.append(xt)
                for m in range(NHC):
                    for (n0, nw) in QT:
                        ps = ps_qk.tile([128, 512], F32, tag="qk")
                        for i, k in enumerate(range(klo, khi)):
                            nc.tensor.matmul(
                                ps[:, :nw],
                                wqk_sb[:, k * 384 + m * 128: k * 384 + (m + 1) * 128],
                                xs[i][:, n0:n0 + nw],
                                start=(i == 0), stop=(i == khi - klo - 1))
                        lt_, lh = lo_dest[m]
                        ht_, hh = hi_dest[m]
                        dlo = lt_[0:64, lh * S + n0: lh * S + n0 + nw]
                        dhi = ht_[64:128, hh * S + n0: hh * S + n0 + nw]
                        if wave == 0:
                            nc.scalar.activation(out=dlo, in_=ps[0:64, :nw],
                                                 func=ACTF.Identity,
                                                 bias=bqk_sb[0:64, m:m + 1])
                            nc.scalar.activation(out=dhi, in_=ps[64:128, :nw],
                                                 func=ACTF.Identity,
                                                 bias=bqk_sb[64:128, m:m + 1])
                        else:
                            nc.vector.tensor_add(dlo, dlo, ps[0:64, :nw])
                            nc.vector.tensor_add(dhi, dhi, ps[64:128, :nw])
                # V natural layout
                for ts in range(TOKT):
                    ps = ps_v.tile([128, NHC * HD], F32, tag="v")
                    for i, k in enumerate(range(klo, khi)):
                        nc.tensor.matmul(
                            ps[:],
                            xs[i][:, ts * 128:(ts + 1) * 128],
                            wv_sb[:, k * 192:(k + 1) * 192],
                            start=(i == 0), stop=(i == khi - klo - 1))
                    vdst = _ap(v32, ts * VST, [[HD + 1, NHC], [1, HD]])
                    vsrc = _ap(ps, 0, [[HD, NHC], [1, HD]])
                    if wave == 0:
                        nc.scalar.activation(out=vdst, in_=vsrc, func=ACTF.Copy)
                    else:
                        nc.vector.tensor_add(vdst, vdst, vsrc)
            # partition-shift the parked high halves into place
            for m in range(NHC):
                ht_, hh = hi_dest[m]
                nc.sync.dma_start(out=ht_[0:64, hh * S:(hh + 1) * S],
                                  in_=ht_[64:128, hh * S:(hh + 1) * S])
            # split V into f32r hi + lo (ones cols stay exact: 1.0 and 0.0)
            if p_split:
                nc.scalar.activation(out=v_hi, in_=v32, func=ACTF.Copy)
                nc.vector.tensor_sub(v_lo, v32, v_hi.bitcast(F32))

        if stop_after == "qkv":
            nc.sync.dma_start(out=dbg["qT"], in_=qT)
            nc.sync.dma_start(out=dbg["kT"], in_=kT)
            nc.sync.dma_start(out=dbg["v"],
                              in_=v_hi.bitcast(F32) if p_split else v_hi)
            return

        late = es.enter_context(tc.tile_pool(name=pfx + "late", bufs=1))
        wp_sb = []
        for j in range(NHC):
            t = late.tile([HD, D], F32, name=f"wp{j}", tag=f"wp{j}")
            nc.sync.dma_start(out=t, in_=wp[j])
            wp_sb.append(t)
        ident = late.tile([128, 128], F32)
        make_identity(nc, ident)
        # bias tables: rows 0-47 rel_w, 48-63 zero, 64-111 rel_h; hi/lo split
        relT = late.tile([112, S], F32R, name="relT", tag="relT")
        nc.gpsimd.dma_start(out=relT[48:64, :], in_=zeros16)
        relTlo = None
        if bias_split:
            relTlo = late.tile([112, S], F32R, name="relTlo", tag="relTlo")
            nc.gpsimd.dma_start(out=relTlo[48:64, :], in_=zeros16)

        # ---------------- phases 2+3: per-head attention ----------------
        with tc.tile_pool(name=pfx + "rel32p", bufs=1) as rel32p, \
             tc.tile_pool(name=pfx + "pTp", bufs=3) as pTp, \
             tc.tile_pool(name=pfx + "lp", bufs=2) as lp, \
             tc.tile_pool(name=pfx + "ps_rel", bufs=2, space="PSUM") as ps_rel, \
             tc.tile_pool(name=pfx + "ps_S", bufs=2, space="PSUM") as ps_S, \
             tc.tile_pool(name=pfx + "ps_O", bufs=2, space="PSUM") as ps_O:
            rel32 = (rel32p.tile([112, S], F32, name="rel32")
                     if bias_split else None)
            for h in range(NHC):
                # rel tables: batches of 10 row-indices share one psum bank;
                # each bank gets exactly two accumulation groups (rel_w rows
                # 0-47 and rel_h rows 64-111, disjoint partitions)
                rel_dst = rel32 if bias_split else relT
                for g in range(5):
                    cnt = 10 if g < 4 else 8
                    ps = ps_rel.tile([128, 480], F32, tag="rel")
                    for i in range(cnt):
                        r = g * 10 + i
                        nc.tensor.matmul(
                            ps[0:48, i * 48:(i + 1) * 48],
                            RwT_sb[:, r * 48:(r + 1) * 48],
                            bass.AP(tensor=qT.tensor,
                                    offset=qT.offset + h * S + r,
                                    ap=[qT[0:64, :].ap[0], [48, 48]]),
                            start=(i == 0), stop=(i == cnt - 1))
                        # out at base partition 64 (col-tiled); the sim's
                        # zero-region bookkeeping mis-indexes partition-offset
                        # psum APs, so skip its group check (single writer per
                        # element; overwrite-vs-accumulate equivalent here)
                        nc.tensor.matmul(
                            ps[64:112, i * 48:(i + 1) * 48],
                            RhT_sb[:, r * 48:(r + 1) * 48],
                            qT[0:64, h * S + r * 48: h * S + (r + 1) * 48],
                            start=(i == 0), stop=(i == cnt - 1),
                            skip_group_check=True)
                    nc.scalar.activation(
                        out=rel_dst[64:112, g * 480: g * 480 + cnt * 48],
                        in_=ps[64:112, 0:cnt * 48], func=ACTF.Copy)
                    wdst = bass.AP(tensor=rel_dst.tensor,
                                   offset=rel_dst.offset + g * 10,
                                   ap=[rel_dst[0:48, :].ap[0], [1, cnt], [48, 48]])
                    wsrc = bass.AP(tensor=ps.tensor, offset=ps.offset,
                                   ap=[ps[0:48, :].ap[0], [48, cnt], [1, 48]])
                    nc.scalar.activation(out=wdst, in_=wsrc, func=ACTF.Copy)
                if bias_split:
                    # hi/lo split (rows 0-47 and 64-111; zero rows preset)
                    for r0, r1 in [(0, 48), (64, 112)]:
                        nc.scalar.activation(out=relT[r0:r1, :],
                                             in_=rel32[r0:r1, :], func=ACTF.Copy)
                        nc.vector.tensor_sub(relTlo[r0:r1, :], rel32[r0:r1, :],
                                             relT[r0:r1, :].bitcast(F32))

                if stop_after == "rel":
                    nc.gpsimd.dma_start(out=dbg["relT"], in_=relT)
                    return

                # attention
                for (q0, qw) in QT:
                    psO = ps_O.tile([HD + 1, 512], F32, tag="o")
                    for kt in range(KT):
                        psS = ps_S.tile([128, 512], F32, tag="s")
                        nc.tensor.matmul(
                            psS[:, :qw],
                            kT[0:64, h * S + kt * 128: h * S + (kt + 1) * 128],
                            qT[0:64, h * S + q0: h * S + q0 + qw],
                            start=True, stop=False)
                        nc.tensor.matmul(
                            psS[:, :qw],
                            Ec_sb[:, kt * 128:(kt + 1) * 128],
                            relT[:, q0:q0 + qw],
                            start=False, stop=not bias_split)
                        if bias_split:
                            nc.tensor.matmul(
                                psS[:, :qw],
                                Ec_sb[:, kt * 128:(kt + 1) * 128],
                                relTlo[:, q0:q0 + qw],
                                start=False, stop=True)
                        pT = pTp.tile([128, 512], F32R if p_split else F32,
                                      tag="p")
                        nc.scalar.activation(out=pT[:, :qw], in_=psS[:, :qw],
                                             func=ACTF.Exp)
                        vsl = slice(kt * VST + h * (HD + 1),
                                    kt * VST + (h + 1) * (HD + 1))
                        nc.tensor.matmul(
                            psO[:, :qw], v_hi[:, vsl], pT[:, :qw],
                            start=(kt == 0),
                            stop=(kt == KT - 1 and not p_split))
                        if p_split:
                            nc.tensor.matmul(
                                psO[:, :qw], v_lo[:, vsl], pT[:, :qw],
                                start=False, stop=(kt == KT - 1))
                    nc.scalar.activation(out=outT[h][:, q0:q0 + qw],
                                         in_=psO[:, :qw], func=ACTF.Copy)

                # softmax denominators -> per-token columns, reciprocal
                psT = ps_O.tile([128, TOKT], F32, tag="t", bufs=2)
                for ts in range(TOKT):
                    nc.tensor.matmul(psT[:, ts:ts + 1],
                                     outT[h][HD:HD + 1, ts * 128:(ts + 1) * 128],
                                     ident[HD:HD + 1, HD:HD + 1],
                                     is_transpose=True,
                                     start=(ts == 0), stop=(ts == TOKT - 1))
                lcols = lp.tile([128, TOKT], F32, tag="lc")
                nc.scalar.activation(out=lcols, in_=psT, func=ACTF.Copy)
                nc.vector.reciprocal(out=reciplc[:, h * TOKT:(h + 1) * TOKT],
                                     in_=lcols)
                if stop_after == "attn1":
                    nc.sync.dma_start(out=dbg["outT"], in_=outT[0])
                    nc.sync.dma_start(out=dbg["recip"], in_=reciplc)
                    return

        if stop_after == "attn3":
            return

        # ---------------- phase 4: output projection ----------------
        with tc.tile_pool(name=pfx + "yw", bufs=2) as yw, \
             tc.tile_pool(name=pfx + "ps_y", bufs=2, space="PSUM") as ps_y:
            for ts in range(TOKT):
                y_acc = yw.tile([128, D], F32, tag="yacc")
                for h in range(NHC):
                    ps = ps_y.tile([128, D], F32, tag="y")
                    for (n0, nw) in [(0, 512), (512, 256)]:
                        nc.tensor.matmul(ps[:, n0:n0 + nw],
                                         outT[h][0:HD, ts * 128:(ts + 1) * 128],
                                         wp_sb[h][:, n0:n0 + nw],
                                         start=True, stop=True)
                    scal = reciplc[:, h * TOKT + ts: h * TOKT + ts + 1]
                    if h == 0:
                        nc.vector.tensor_scalar_mul(out=y_acc, in0=ps[:],
                                                    scalar1=scal)
                    else:
                        z = yw.tile([128, D], F32, tag="ztmp", bufs=1)
                        nc.vector.tensor_scalar_mul(out=z, in0=ps[:], scalar1=scal)
                        nc.vector.tensor_add(y_acc, y_acc, z)
                nc.sync.dma_start(out=y[ts * 128:(ts + 1) * 128, :], in_=y_acc)


def build_nc(num_devices=N_CORES, p_split=True, bias_split=True,
             stop_after="full", reps=1):
    nc = bacc.Bacc("TRN2", target_bir_lowering=False, debug=False,
                   num_devices=num_devices)
    aps = (
        nc.dram_tensor("xT", [D, S], F32, kind="ExternalInput").ap(),
        nc.dram_tensor("wqk", [D, 2 * NHC * HD], F32, kind="ExternalInput").ap(),
        nc.dram_tensor("bqk", [128, NHC], F32, kind="ExternalInput").ap(),
        nc.dram_tensor("wv", [D, NHC * HD], F32, kind="ExternalInput").ap(),
        nc.dram_tensor("wp", [NHC, HD, D], F32, kind="ExternalInput").ap(),
        nc.dram_tensor("RhT", [HD, S], F32, kind="ExternalInput").ap(),
        nc.dram_tensor("RwT", [HD, S], F32, kind="ExternalInput").ap(),
        nc.dram_tensor("Ecomb", [112, S], F32, kind="ExternalInput").ap(),
        nc.dram_tensor("zeros16", [16, S], F32, kind="ExternalInput").ap(),
        nc.dram_tensor("y", [S, D], F32, kind="ExternalOutput").ap(),
    )
    dbg = {}
    if stop_after == "qkv":
        dbg["qT"] = nc.dram_tensor("dbg_qT", [HD, NHC * S], F32,
                                   kind="ExternalOutput").ap()
        dbg["kT"] = nc.dram_tensor("dbg_kT", [HD, NHC * S], F32,
                                   kind="ExternalOutput").ap()
        dbg["v"] = nc.dram_tensor("dbg_v", [128, TOKT * VST], F32,
                                  kind="ExternalOutput").ap()
    elif stop_after == "rel":
        dbg["relT"] = nc.dram_tensor("dbg_relT", [112, S], F32,
                                     kind="ExternalOutput").ap()
    elif stop_after == "attn1":
        dbg["outT"] = nc.dram_tensor("dbg_outT", [HD + 1, S], F32,
                                     kind="ExternalOutput").ap()
        dbg["recip"] = nc.dram_tensor("dbg_recip", [128, NHC * TOKT], F32,
                                      kind="ExternalOutput").ap()
    with tile.TileContext(nc) as tc:
        for rep in range(reps):
            _emit(tc, nc, aps, pfx=f"r{rep}_" if reps > 1 else "",
                  p_split=p_split, bias_split=bias_split,
                  stop_after=stop_after, dbg=dbg)
    nc.compile()
    return nc


def prep_core_inputs(c, x, qkv_w, qkv_b, proj_w, rel_pos_h, rel_pos_w):
    b = c // 4
    heads = [3 * (c % 4) + j for j in range(NHC)]
    f32 = np.float32
    xT = np.ascontiguousarray(np.asarray(x, f32)[b].reshape(S, D).T)
    qkv_w = np.asarray(qkv_w, f32)
    qkv_b = np.asarray(qkv_b, f32)
    wq = np.concatenate([qkv_w[:, h * HD:(h + 1) * HD] for h in heads], 1) * f32(SCALE)
    wk = np.concatenate([qkv_w[:, D + h * HD:D + (h + 1) * HD] for h in heads], 1)
    wqk = np.ascontiguousarray(np.concatenate([wq, wk], 1))
    bq = [qkv_b[h * HD:(h + 1) * HD] * f32(SCALE) for h in heads]
    bk = [qkv_b[D + h * HD:D + (h + 1) * HD] for h in heads]
    # per-M-tile half-stacked biases: [q0|q1], [q2|k0], [k1|k2]
    halves = [bq[0], bq[1], bq[2], bk[0], bk[1], bk[2]]
    bqk = np.stack([np.concatenate([halves[2 * m], halves[2 * m + 1]])
                    for m in range(NHC)], 1).astype(f32)
    wv = np.ascontiguousarray(
        np.concatenate([qkv_w[:, 2 * D + h * HD:2 * D + (h + 1) * HD]
                        for h in heads], 1))
    wp = np.ascontiguousarray(
        np.stack([np.asarray(proj_w, f32)[h * HD:(h + 1) * HD, :]
                  for h in heads], 0))
    coords = np.arange(H)[:, None] - np.arange(H)[None, :] + (H - 1)
    Rh = np.asarray(rel_pos_h, f32)[coords]      # [hq, hk, c]
    Rw = np.asarray(rel_pos_w, f32)[coords]      # [wq, wk, c]
    # The reference builds the rel bias from the UNSCALED q; we fold `SCALE`
    # into wq/bq, so fold the exact inverse (8.0) into the rel tables.
    inv = f32(1.0 / SCALE)
    RhT = np.ascontiguousarray(np.transpose(Rh, (2, 0, 1)).reshape(HD, S)) * inv
    RwT = np.ascontiguousarray(np.transpose(Rw, (2, 0, 1)).reshape(HD, S)) * inv
    E = np.zeros((112, S), f32)
    kk = np.arange(S)
    E[kk % W, kk] = 1.0           # rel_w one-hot rows 0..47
    E[64 + kk // W, kk] = 1.0     # rel_h one-hot rows 64..111
    return {"xT": xT, "wqk": wqk, "bqk": bqk, "wv": wv, "wp": wp,
            "RhT": RhT, "RwT": RwT, "Ecomb": E,
            "zeros16": np.zeros((16, S), f32)}


_NC_CACHE = {}


def _get_nc(**kw):
    key = str(sorted(kw.items()))
    if key not in _NC_CACHE:
        _NC_CACHE[key] = build_nc(**kw)
    return _NC_CACHE[key]


def gather_output(ys, qkv_b, proj_w, proj_b):
    f32 = np.float32
    bp_eff = (np.asarray(proj_b, f32)
              + np.asarray(qkv_b, f32)[2 * D:] @ np.asarray(proj_w, f32))
    out = np.empty((B, H, W, D), f32)
    for b in range(B):
        acc = ys[4 * b].copy()
        for j in range(1, 4):
            acc += ys[4 * b + j]
        acc += bp_eff
        out[b] = acc.reshape(H, W, D)
    return out


def kernel(x, qkv_w, qkv_b, proj_w, proj_b, rel_pos_h, rel_pos_w):
    import os
    from concourse.bass_utils import run_bass_kernel_spmd
    nc = _get_nc(p_split=os.environ.get("KERNEL_SAFE", "0") != "1")
    in_maps = [prep_core_inputs(c, x, qkv_w, qkv_b, proj_w, rel_pos_h, rel_pos_w)
               for c in range(N_CORES)]
    res = run_bass_kernel_spmd(nc, in_maps, core_ids=list(range(N_CORES)))
    ys = [res.results[c]["y"] for c in range(N_CORES)]
    return gather_output(ys, qkv_b, proj_w, proj_b)



# revision 7
# speedup vs baseline: 4672.9459x; 4672.9459x over previous
"""Trainium2 Bass kernel for nn_Attention_15857019256917 (ViTDet-style attention
with decomposed relative position bias).

Sharding: data-parallel over B (2) x head-parallel (12 heads -> 4 groups of 3)
= 8 cores. Each core computes 3 heads of attention for one batch element plus
its partial output projection (rows of proj_w for its heads); the host sums the
4 partials per batch element (row-parallel linear unshard) and adds the bias
(with the v-bias folded in exactly: P@(V+1 bv^T)/l = PV/l + bv^T).

bf16 datapath (rel-err gate is 2e-2; measured ~1e-3): all matmul operands are
bf16 (1 PE cycle/col vs fp32's 4), PSUM accumulation stays fp32. Engine
assignment keeps ACT free for the softmax exp (the throughput floor):
  PE:   qkv projection, rel-table build, QK^T+bias logits, PV, 1/l broadcast,
        output projection
  ACT:  exp(S^T) only
  DVE:  qkv bias-add copies from PSUM, reciprocal, PV normalization multiply
  Pool: V interleave copies, rel-table copies from PSUM
  DMA:  tensor loads, parked-half partition shifts, y store (direct from PSUM)
The PV matmul for k-tile kt is emitted after the logit matmuls of kt+1 so the
exp of kt runs under them and PE never waits on ACT.
"""
import sys

sys.path.insert(0, "/opt/trn_rl_repo")

import numpy as np

import concourse.bass as bass
import concourse.bacc as bacc
import concourse.tile as tile
from concourse import mybir

F32 = mybir.dt.float32
BF16 = mybir.dt.bfloat16
ACTF = mybir.ActivationFunctionType

B, H, W, D = 2, 48, 48, 768
NH, HD = 12, 64
S = H * W                      # 2304
SCALE = HD ** -0.5
N_CORES = 8
NHC = 3                        # heads per core
KT = S // 128                  # 18 key tiles
TOKT = S // 128                # 18 token tiles
KCH = D // 128                 # 6 contraction chunks
QT = [(0, 512), (512, 512), (1024, 512), (1536, 512), (2048, 256)]
VST = NHC * (HD + 1)           # 195: per-ktile V layout [v_h0|1|v_h1|1|v_h2|1]


def _ap(t, off_elems, dims):
    """Raw AP on tile t: partition dim copied, free dims = [[step, count], ...]."""
    return bass.AP(tensor=t.tensor, offset=t.offset + off_elems, ap=[t.ap[0]] + dims)


def _emit(tc, nc, aps, pfx=""):
    xT, wqk, bqk, wv, wp, RhT, RwT, Ec, y = aps
    from contextlib import ExitStack

    with ExitStack() as es:
        consts = es.enter_context(tc.tile_pool(name=pfx + "consts", bufs=1))
        big = es.enter_context(tc.tile_pool(name=pfx + "big", bufs=1))

        RhT_sb = consts.tile([HD, S], BF16)
        nc.sync.dma_start(out=RhT_sb, in_=RhT)
        RwT_sb = consts.tile([HD, S], BF16)
        nc.sync.dma_start(out=RwT_sb, in_=RwT)
        Ec_sb = consts.tile([112, S], BF16)
        nc.sync.dma_start(out=Ec_sb, in_=Ec)

        qT = big.tile([128, NHC * S], BF16)
        kT = big.tile([128, NHC * S], BF16)
        outT = [big.tile([HD, S], BF16, name=f"outT{j}", tag=f"outT{j}")
                for j in range(NHC)]
        v = big.tile([128, TOKT * VST], BF16, name="v", tag="v")

        # ---------------- phase 1: qkv projections ----------------
        with tc.tile_pool(name=pfx + "ph1", bufs=1) as ph1, \
             tc.tile_pool(name=pfx + "ps_qk", bufs=2, space="PSUM") as ps_qk, \
             tc.tile_pool(name=pfx + "ps_v", bufs=2, space="PSUM") as ps_v:
            wqk_sb = ph1.tile([128, KCH * 2 * NHC * HD], BF16)  # [128, 6*384]
            wv_sb = ph1.tile([128, KCH * NHC * HD], BF16)       # [128, 6*192]
            bqk_sb = ph1.tile([128, NHC], F32)                  # half-stacked biases
            nc.sync.dma_start(out=bqk_sb, in_=bqk)
            nc.vector.memset(_ap(v, HD, [[VST, TOKT], [HD + 1, NHC]]), 1.0)
            xs = []
            for k in range(KCH):
                nc.sync.dma_start(out=wqk_sb[:, k * 384:(k + 1) * 384],
                                  in_=wqk[k * 128:(k + 1) * 128, :])
                nc.sync.dma_start(out=wv_sb[:, k * 192:(k + 1) * 192],
                                  in_=wv[k * 128:(k + 1) * 128, :])
                xt = ph1.tile([128, S], BF16, name=f"x{k}", tag=f"x{k}")
                nc.sync.dma_start(out=xt, in_=xT[k * 128:(k + 1) * 128, :])
                xs.append(xt)

            # M-tiles (128 rows = two 64-channel halves):
            #   T0=[q0|q1]  T1=[q2|k0]  T2=[k1|k2]
            # low halves copy straight to rows 0-63 of their dest tensor; high
            # halves park in the dest tensor's padding rows 64-127 (same column
            # range), then an intra-tensor DMA partition-shifts them down.
            lo_dest = [(qT, 0), (qT, 2), (kT, 1)]
            hi_dest = [(qT, 1), (kT, 0), (kT, 2)]
            for m in range(NHC):
                for (n0, nw) in QT:
                    ps = ps_qk.tile([128, 512], F32, tag="qk")
                    for k in range(KCH):
                        nc.tensor.matmul(
                            ps[:, :nw],
                            wqk_sb[:, k * 384 + m * 128: k * 384 + (m + 1) * 128],
                            xs[k][:, n0:n0 + nw],
                            start=(k == 0), stop=(k == KCH - 1))
                    lt_, lh = lo_dest[m]
                    ht_, hh = hi_dest[m]
                    dlo = lt_[0:64, lh * S + n0: lh * S + n0 + nw]
                    dhi = ht_[64:128, hh * S + n0: hh * S + n0 + nw]
                    nc.vector.tensor_scalar_add(dlo, ps[0:64, :nw],
                                                bqk_sb[0:64, m:m + 1])
                    nc.vector.tensor_scalar_add(dhi, ps[64:128, :nw],
                                                bqk_sb[64:128, m:m + 1])
            # V natural layout
            for ts in range(TOKT):
                ps = ps_v.tile([128, NHC * HD], F32, tag="v")
                for k in range(KCH):
                    nc.tensor.matmul(
                        ps[:],
                        xs[k][:, ts * 128:(ts + 1) * 128],
                        wv_sb[:, k * 192:(k + 1) * 192],
                        start=(k == 0), stop=(k == KCH - 1))
                vdst = _ap(v, ts * VST, [[HD + 1, NHC], [1, HD]])
                vsrc = _ap(ps, 0, [[HD, NHC], [1, HD]])
                nc.vector.tensor_copy(vdst, vsrc)
            # partition-shift the parked high halves into place
            for m in range(NHC):
                ht_, hh = hi_dest[m]
                nc.sync.dma_start(out=ht_[0:64, hh * S:(hh + 1) * S],
                                  in_=ht_[64:128, hh * S:(hh + 1) * S])

        late = es.enter_context(tc.tile_pool(name=pfx + "late", bufs=1))
        wp_sb = []
        for j in range(NHC):
            t = late.tile([HD, D], BF16, name=f"wp{j}", tag=f"wp{j}")
            nc.sync.dma_start(out=t, in_=wp[j])
            wp_sb.append(t)
        # bias tables: rows 0-47 rel_w, 48-63 zero, 64-111 rel_h
        relT = late.tile([112, S], BF16, name="relT", tag="relT")
        # zero rows 48-63 (engine start partition must be 0/32/64/96; rows
        # 32-47 get overwritten with real rel_w data per head)
        nc.vector.memset(relT[32:64, :], 0.0)
        ones64 = late.tile([1, HD], BF16)
        nc.vector.memset(ones64, 1.0)

        # ---------------- phases 2+3: per-head attention ----------------
        with tc.tile_pool(name=pfx + "pTp", bufs=3) as pTp, \
             tc.tile_pool(name=pfx + "lrp", bufs=2) as lrp, \
             tc.tile_pool(name=pfx + "ps_rel", bufs=2, space="PSUM") as ps_rel, \
             tc.tile_pool(name=pfx + "ps_S", bufs=2, space="PSUM") as ps_S, \
             tc.tile_pool(name=pfx + "ps_O", bufs=2, space="PSUM") as ps_O, \
             tc.tile_pool(name=pfx + "ps_bc", bufs=1, space="PSUM") as ps_bc:
            for h in range(NHC):
                # rel tables: batches of 10 row-indices share one psum bank;
                # each bank gets exactly two accumulation groups (rel_w rows
                # 0-47 and rel_h rows 64-111, disjoint partitions)
                for g in range(5):
                    cnt = 10 if g < 4 else 8
                    ps = ps_rel.tile([128, 480], F32, tag="rel")
                    for i in range(cnt):
                        r = g * 10 + i
                        nc.tensor.matmul(
                            ps[0:48, i * 48:(i + 1) * 48],
                            RwT_sb[:, r * 48:(r + 1) * 48],
                            bass.AP(tensor=qT.tensor,
                                    offset=qT.offset + h * S + r,
                                    ap=[qT[0:64, :].ap[0], [48, 48]]),
                            start=(i == 0), stop=(i == cnt - 1))
                        # out at base partition 64 (col-tiled); the sim's
                        # zero-region bookkeeping mis-indexes partition-offset
                        # psum APs, so skip its group check (single writer per
                        # element; overwrite-vs-accumulate equivalent here)
                        nc.tensor.matmul(
                            ps[64:112, i * 48:(i + 1) * 48],
                            RhT_sb[:, r * 48:(r + 1) * 48],
                            qT[0:64, h * S + r * 48: h * S + (r + 1) * 48],
                            start=(i == 0), stop=(i == cnt - 1),
                            skip_group_check=True)
                    nc.vector.tensor_copy(
                        relT[64:112, g * 480: g * 480 + cnt * 48],
                        ps[64:112, 0:cnt * 48])
                    wdst = bass.AP(tensor=relT.tensor,
                                   offset=relT.offset + g * 10,
                                   ap=[relT[0:48, :].ap[0], [1, cnt], [48, 48]])
                    wsrc = bass.AP(tensor=ps.tensor, offset=ps.offset,
                                   ap=[ps[0:48, :].ap[0], [48, cnt], [1, 48]])
                    nc.vector.tensor_copy(wdst, wsrc)

                # attention; PV trails the logit matmuls by one k-tile so the
                # exp overlaps PE work
                for (q0, qw) in QT:
                    psO = ps_O.tile([HD + 1, 512], F32, tag="o")
                    pts = []
                    for kt in range(KT):
                        psS = ps_S.tile([128, 512], F32, tag="s")
                        nc.tensor.matmul(
                            psS[:, :qw],
                            kT[0:64, h * S + kt * 128: h * S + (kt + 1) * 128],
                            qT[0:64, h * S + q0: h * S + q0 + qw],
                            start=True, stop=False)
                        nc.tensor.matmul(
                            psS[:, :qw],
                            Ec_sb[:, kt * 128:(kt + 1) * 128],
                            relT[:, q0:q0 + qw],
                            start=False, stop=True)
                        pT = pTp.tile([128, 512], BF16, tag="p")
                        nc.scalar.activation(out=pT[:, :qw], in_=psS[:, :qw],
                                             func=ACTF.Exp)
                        pts.append(pT)
                        if kt >= 1:
                            vsl = slice((kt - 1) * VST + h * (HD + 1),
                                        (kt - 1) * VST + (h + 1) * (HD + 1))
                            nc.tensor.matmul(
                                psO[:, :qw], v[:, vsl], pts[kt - 1][:, :qw],
                                start=(kt == 1), stop=False)
                    vsl = slice((KT - 1) * VST + h * (HD + 1),
                                (KT - 1) * VST + (h + 1) * (HD + 1))
                    nc.tensor.matmul(
                        psO[:, :qw], v[:, vsl], pts[KT - 1][:, :qw],
                        start=False, stop=True)

                    # normalize: outT = psO[0:64] * broadcast(1/l)
                    lr = lrp.tile([1, 512], BF16, tag="lr")
                    with nc.allow_low_precision(reason="1/l in bf16: uniform "
                                                "2^-9 scale noise, gate is 2e-2"):
                        nc.vector.reciprocal(out=lr[:, :qw],
                                             in_=psO[HD:HD + 1, :qw])
                    psB = ps_bc.tile([HD, 512], F32, tag="bc")
                    nc.tensor.matmul(psB[:, :qw], ones64, lr[0:1, :qw],
                                     start=True, stop=True)
                    # DVE may read only one PSUM operand per instruction:
                    # stage the broadcast 1/l in SBUF, then multiply
                    lrb = lrp.tile([HD, 512], BF16, tag="lrb")
                    nc.vector.tensor_copy(lrb[:, :qw], psB[:, :qw])
                    nc.vector.tensor_mul(outT[h][:, q0:q0 + qw],
                                         psO[0:HD, :qw], lrb[:, :qw])

        # ---------------- phase 4: output projection ----------------
        with tc.tile_pool(name=pfx + "ps_y", bufs=2, space="PSUM") as ps_y, \
             tc.tile_pool(name=pfx + "yw", bufs=2) as yw:
            for ts in range(TOKT):
                psA = ps_y.tile([128, 512], F32, tag="yA")
                psB2 = ps_y.tile([128, 256], F32, tag="yB")
                for h in range(NHC):
                    src = outT[h][:, ts * 128:(ts + 1) * 128]
                    nc.tensor.matmul(psA, src, wp_sb[h][:, 0:512],
                                     start=(h == 0), stop=(h == NHC - 1))
                    nc.tensor.matmul(psB2, src, wp_sb[h][:, 512:768],
                                     start=(h == 0), stop=(h == NHC - 1))
                y_sb = yw.tile([128, D], F32, tag="ysb")
                nc.vector.tensor_copy(y_sb[:, 0:512], psA)
                nc.vector.tensor_copy(y_sb[:, 512:768], psB2)
                nc.sync.dma_start(out=y[ts * 128:(ts + 1) * 128, :], in_=y_sb)


def build_nc(num_devices=N_CORES, reps=1):
    nc = bacc.Bacc("TRN2", target_bir_lowering=False, debug=False,
                   num_devices=num_devices)
    aps = (
        nc.dram_tensor("xT", [D, S], BF16, kind="ExternalInput").ap(),
        nc.dram_tensor("wqk", [D, 2 * NHC * HD], BF16, kind="ExternalInput").ap(),
        nc.dram_tensor("bqk", [128, NHC], F32, kind="ExternalInput").ap(),
        nc.dram_tensor("wv", [D, NHC * HD], BF16, kind="ExternalInput").ap(),
        nc.dram_tensor("wp", [NHC, HD, D], BF16, kind="ExternalInput").ap(),
        nc.dram_tensor("RhT", [HD, S], BF16, kind="ExternalInput").ap(),
        nc.dram_tensor("RwT", [HD, S], BF16, kind="ExternalInput").ap(),
        nc.dram_tensor("Ec", [112, S], BF16, kind="ExternalInput").ap(),
        nc.dram_tensor("y", [S, D], F32, kind="ExternalOutput").ap(),
    )
    with tile.TileContext(nc) as tc:
        for rep in range(reps):
            _emit(tc, nc, aps, pfx=f"r{rep}_" if reps > 1 else "")
    nc.compile()
    return nc


def prep_core_inputs(c, x, qkv_w, qkv_b, proj_w, rel_pos_h, rel_pos_w):
    bf16 = mybir.dt.np(BF16)
    b = c // 4
    heads = [3 * (c % 4) + j for j in range(NHC)]
    f32 = np.float32
    xTa = np.ascontiguousarray(np.asarray(x, f32)[b].reshape(S, D).T).astype(bf16)
    qkv_w = np.asarray(qkv_w, f32)
    qkv_b = np.asarray(qkv_b, f32)
    wq = np.concatenate([qkv_w[:, h * HD:(h + 1) * HD] for h in heads], 1) * f32(SCALE)
    wk = np.concatenate([qkv_w[:, D + h * HD:D + (h + 1) * HD] for h in heads], 1)
    wqka = np.ascontiguousarray(np.concatenate([wq, wk], 1)).astype(bf16)
    bq = [qkv_b[h * HD:(h + 1) * HD] * f32(SCALE) for h in heads]
    bk = [qkv_b[D + h * HD:D + (h + 1) * HD] for h in heads]
    # per-M-tile half-stacked biases: [q0|q1], [q2|k0], [k1|k2]
    halves = [bq[0], bq[1], bq[2], bk[0], bk[1], bk[2]]
    bqka = np.stack([np.concatenate([halves[2 * m], halves[2 * m + 1]])
                     for m in range(NHC)], 1).astype(f32)
    wva = np.ascontiguousarray(
        np.concatenate([qkv_w[:, 2 * D + h * HD:2 * D + (h + 1) * HD]
                        for h in heads], 1)).astype(bf16)
    wpa = np.ascontiguousarray(
        np.stack([np.asarray(proj_w, f32)[h * HD:(h + 1) * HD, :]
                  for h in heads], 0)).astype(bf16)
    coords = np.arange(H)[:, None] - np.arange(H)[None, :] + (H - 1)
    Rh = np.asarray(rel_pos_h, f32)[coords]      # [hq, hk, c]
    Rw = np.asarray(rel_pos_w, f32)[coords]      # [wq, wk, c]
    # The reference builds the rel bias from the UNSCALED q; we fold `SCALE`
    # into wq/bq, so fold the exact inverse (8.0) into the rel tables.
    inv = f32(1.0 / SCALE)
    RhTa = (np.ascontiguousarray(np.transpose(Rh, (2, 0, 1)).reshape(HD, S))
            * inv).astype(bf16)
    RwTa = (np.ascontiguousarray(np.transpose(Rw, (2, 0, 1)).reshape(HD, S))
            * inv).astype(bf16)
    E = np.zeros((112, S), bf16)
    kk = np.arange(S)
    E[kk % W, kk] = 1.0           # rel_w one-hot rows 0..47
    E[64 + kk // W, kk] = 1.0     # rel_h one-hot rows 64..111
    return {"xT": xTa, "wqk": wqka, "bqk": bqka, "wv": wva, "wp": wpa,
            "RhT": RhTa, "RwT": RwTa, "Ec": E}


_NC_CACHE = {}


def _get_nc(**kw):
    key = str(sorted(kw.items()))
    if key not in _NC_CACHE:
        _NC_CACHE[key] = build_nc(**kw)
    return _NC_CACHE[key]


def gather_output(ys, qkv_b, proj_w, proj_b):
    f32 = np.float32
    bp_eff = (np.asarray(proj_b, f32)
              + np.asarray(qkv_b, f32)[2 * D:] @ np.asarray(proj_w, f32))
    out = np.empty((B, H, W, D), f32)
    for b in range(B):
        acc = ys[4 * b].copy()
        for j in range(1, 4):
            acc += ys[4 * b + j]
        acc += bp_eff
        out[b] = acc.reshape(H, W, D)
    return out


def kernel(x, qkv_w, qkv_b, proj_w, proj_b, rel_pos_h, rel_pos_w):
    from concourse.bass_utils import run_bass_kernel_spmd
    nc = _get_nc()
    in_maps = [prep_core_inputs(c, x, qkv_w, qkv_b, proj_w, rel_pos_h, rel_pos_w)
               for c in range(N_CORES)]
    res = run_bass_kernel_spmd(nc, in_maps, core_ids=list(range(N_CORES)))
    ys = [res.results[c]["y"] for c in range(N_CORES)]
    return gather_output(ys, qkv_b, proj_w, proj_b)


# revision 21
# speedup vs baseline: 5871.3658x; 1.2565x over previous
"""Trainium2 Bass kernel for nn_Attention_15857019256917 (ViTDet-style attention
with decomposed relative position bias).

Sharding: data-parallel over B (2) x head-parallel (12 heads -> 4 groups of 3)
= 8 cores. Each core computes 3 heads of attention for one batch element plus
its partial output projection (rows of proj_w for its heads); the host sums the
4 partials per batch element (row-parallel linear unshard) and adds the bias
(with the v-bias folded in exactly: P@(V+1 bv^T)/l = PV/l + bv^T).

bf16 datapath (rel-err gate is 2e-2; measured ~1e-3): all matmul operands are
bf16 (1 PE cycle/col vs fp32's 4), PSUM accumulation stays fp32. Engine
assignment keeps ACT free for the softmax exp (the throughput floor):
  PE:   qkv projection, rel-table build, QK^T+bias logits, PV, 1/l broadcast,
        output projection
  ACT:  exp(S^T) only
  DVE:  qkv bias-add copies from PSUM, reciprocal, PV normalization multiply
  Pool: V interleave copies, rel-table copies from PSUM
  DMA:  tensor loads, parked-half partition shifts, y store (direct from PSUM)
The PV matmul for k-tile kt is emitted after the logit matmuls of kt+1 so the
exp of kt runs under them and PE never waits on ACT.
"""
import sys

sys.path.insert(0, "/opt/trn_rl_repo")

import numpy as np

import concourse.bass as bass
import concourse.bacc as bacc
import concourse.tile as tile
from concourse import mybir

F32 = mybir.dt.float32
BF16 = mybir.dt.bfloat16
ACTF = mybir.ActivationFunctionType

B, H, W, D = 2, 48, 48, 768
NH, HD = 12, 64
S = H * W                      # 2304
SCALE = HD ** -0.5
N_CORES = 8
NHC = 3                        # heads per core
KT = S // 128                  # 18 key tiles
TOKT = S // 128                # 18 token tiles
KCH = D // 128                 # 6 contraction chunks
QT = [(0, 512), (512, 512), (1024, 512), (1536, 512), (2048, 256)]
VST = NHC * (HD + 1)           # 195: per-ktile V layout [v_h0|1|v_h1|1|v_h2|1]


def _ap(t, off_elems, dims):
    """Raw AP on tile t: partition dim copied, free dims = [[step, count], ...]."""
    return bass.AP(tensor=t.tensor, offset=t.offset + off_elems, ap=[t.ap[0]] + dims)


def _emit(tc, nc, aps, pfx=""):
    xT, wqk, bqk, wv, wp, RhT, RwT, Estat, y = aps
    from contextlib import ExitStack

    with ExitStack() as es:
        consts = es.enter_context(tc.tile_pool(name=pfx + "consts", bufs=1))
        big = es.enter_context(tc.tile_pool(name=pfx + "big", bufs=1))

        RhT_sb = consts.tile([HD, S], BF16)
        nc.gpsimd.dma_start(out=RhT_sb, in_=RhT)
        RwT_sb = consts.tile([HD, S], BF16)
        nc.gpsimd.dma_start(out=RwT_sb, in_=RwT)

        # Augmented operands for the single-matmul logits (contraction rows
        # 0-119): rows 0-63 q/k, rows 64-111 rel_w table / one-hot(k%48),
        # rows 112-119 rel_h window / one-hot(k//48 - pair_base). QaugA/B
        # alternate per k-tile pair so the window DMA for pair p+1 never
        # touches the tile pair p's matmuls read. Rows 120-127 are spare
        # (phase-1 parking uses 64-127 transiently, before rel rows load).
        Kaug = big.tile([128, NHC * S], BF16, name="Kaug", tag="Kaug")
        QaugA = big.tile([128, NHC * S], BF16, name="QaugA", tag="QaugA")
        QaugB = big.tile([128, NHC * S], BF16, name="QaugB", tag="QaugB")
        relh = big.tile([48, NHC * S], BF16, name="relh", tag="relh")
        outT = [big.tile([HD, S], BF16, name=f"outT{j}", tag=f"outT{j}")
                for j in range(NHC)]
        v = big.tile([128, TOKT * VST], BF16, name="v", tag="v")

        # ---------------- phase 1: qkv projections ----------------
        with tc.tile_pool(name=pfx + "ph1", bufs=1) as ph1, \
             tc.tile_pool(name=pfx + "ps_qk", bufs=2, space="PSUM") as ps_qk, \
             tc.tile_pool(name=pfx + "ps_v", bufs=2, space="PSUM") as ps_v:
            wqk_sb = ph1.tile([128, KCH * 2 * NHC * HD], BF16)  # [128, 6*384]
            wv_sb = ph1.tile([128, KCH * NHC * HD], BF16)       # [128, 6*192]
            bqk_sb = ph1.tile([128, NHC], F32)                  # half-stacked biases
            nc.sync.dma_start(out=bqk_sb, in_=bqk)
            nc.vector.memset(_ap(v, HD, [[VST, TOKT], [HD + 1, NHC]]), 1.0)
            xs = []
            for k in range(KCH):
                nc.sync.dma_start(out=wqk_sb[:, k * 384:(k + 1) * 384],
                                  in_=wqk[k * 128:(k + 1) * 128, :])
                nc.sync.dma_start(out=wv_sb[:, k * 192:(k + 1) * 192],
                                  in_=wv[k * 128:(k + 1) * 128, :])
                xt = ph1.tile([128, S], BF16, name=f"x{k}", tag=f"x{k}")
                nc.sync.dma_start(out=xt, in_=xT[k * 128:(k + 1) * 128, :])
                xs.append(xt)

            # M-tiles (128 rows = two 64-channel halves):
            #   T0=[q0|q1]  T1=[q2|k0]  T2=[k1|k2]
            # low halves copy straight to rows 0-63 of their dest tensor; high
            # halves park in the dest tensor's padding rows 64-127 (same column
            # range), then an intra-tensor DMA partition-shifts them down.
            # low halves go straight to rows 0-63; high halves park in
            # QaugB rows 64-127 (free until the rel_w rows are built), so
            # Kaug's static one-hot rows can load immediately
            lo_dest = [(QaugA, 0), (QaugA, 2), (Kaug, 1)]
            hi_dest = [(QaugA, 1), (Kaug, 0), (Kaug, 2)]
            for j in range(NHC):
                nc.gpsimd.dma_start(out=Kaug[64:112, j * S:(j + 1) * S],
                                    in_=Estat[0:48, :])
                nc.gpsimd.dma_start(out=Kaug[112:120, j * S:(j + 1) * S],
                                    in_=Estat[48:56, :])
            for m in range(NHC):
                for (n0, nw) in QT:
                    ps = ps_qk.tile([128, 512], F32, tag="qk")
                    for k in range(KCH):
                        nc.tensor.matmul(
                            ps[:, :nw],
                            wqk_sb[:, k * 384 + m * 128: k * 384 + (m + 1) * 128],
                            xs[k][:, n0:n0 + nw],
                            start=(k == 0), stop=(k == KCH - 1))
                    lt_, lh = lo_dest[m]
                    ht_, hh = hi_dest[m]
                    dlo = lt_[0:64, lh * S + n0: lh * S + n0 + nw]
                    dhi = QaugB[64:128, hh * S + n0: hh * S + n0 + nw]
                    nc.vector.tensor_scalar_add(dlo, ps[0:64, :nw],
                                                bqk_sb[0:64, m:m + 1])
                    nc.vector.tensor_scalar_add(dhi, ps[64:128, :nw],
                                                bqk_sb[64:128, m:m + 1])
            # V natural layout
            for ts in range(TOKT):
                ps = ps_v.tile([128, NHC * HD], F32, tag="v")
                for k in range(KCH):
                    nc.tensor.matmul(
                        ps[:],
                        xs[k][:, ts * 128:(ts + 1) * 128],
                        wv_sb[:, k * 192:(k + 1) * 192],
                        start=(k == 0), stop=(k == KCH - 1))
                vdst = _ap(v, ts * VST, [[HD + 1, NHC], [1, HD]])
                vsrc = _ap(ps, 0, [[HD, NHC], [1, HD]])
                nc.vector.tensor_copy(vdst, vsrc)
            # partition-shift the parked high halves into place
            for m in range(NHC):
                ht_, hh = hi_dest[m]
                nc.sync.dma_start(out=ht_[0:64, hh * S:(hh + 1) * S],
                                  in_=QaugB[64:128, hh * S:(hh + 1) * S])
            # q lives in both Qaug variants
            nc.vector.tensor_copy(QaugB[0:64, :], QaugA[0:64, :])

        late = es.enter_context(tc.tile_pool(name=pfx + "late", bufs=1))
        wp_sb = []
        for j in range(NHC):
            t = late.tile([HD, D], BF16, name=f"wp{j}", tag=f"wp{j}")
            nc.gpsimd.dma_start(out=t, in_=wp[j])
            wp_sb.append(t)
        ones64 = late.tile([1, HD], BF16)
        nc.vector.memset(ones64, 1.0)

        # ---------------- phases 2+3: attention ----------------
        NQT = len(QT)

        def fill_window(h, p):
            """Stage rel_h rows base..base+7 for k-tile pair p into the
            pair-parity Qaug window rows 112-119 (one DMA; clamped rows
            leave stale-but-finite values under zero one-hot columns)."""
            Qx = QaugA if p % 2 == 0 else QaugB
            base = (256 * p) // 48
            n = min(8, 48 - base)
            nc.gpsimd.dma_start(out=Qx[112:112 + n, h * S:(h + 1) * S],
                                in_=relh[base:base + n, h * S:(h + 1) * S])

        # rel tables for ALL heads up front (PE-only, keeps ACT fed during
        # attention): rel_h lands at psum rows 0-47 (copied to the relh
        # staging tile), rel_w at rows 64-111 (copied into both Qaug variants)
        with tc.tile_pool(name=pfx + "ps_rel", bufs=2, space="PSUM") as ps_rel:
            for h in range(NHC):
                for g in range(5):
                    cnt = 10 if g < 4 else 8
                    ps = ps_rel.tile([128, 480], F32, tag="rel")
                    for i in range(cnt):
                        r = g * 10 + i
                        nc.tensor.matmul(
                            ps[0:48, i * 48:(i + 1) * 48],
                            RhT_sb[:, r * 48:(r + 1) * 48],
                            QaugA[0:64, h * S + r * 48: h * S + (r + 1) * 48],
                            start=(i == 0), stop=(i == cnt - 1))
                        # out at base partition 64 (col-tiled); the sim's
                        # zero-region bookkeeping mis-indexes partition-offset
                        # psum APs, so skip its group check (single writer)
                        nc.tensor.matmul(
                            ps[64:112, i * 48:(i + 1) * 48],
                            RwT_sb[:, r * 48:(r + 1) * 48],
                            bass.AP(tensor=QaugA.tensor,
                                    offset=QaugA.offset + h * S + r,
                                    ap=[QaugA[0:64, :].ap[0], [48, 48]]),
                            start=(i == 0), stop=(i == cnt - 1),
                            skip_group_check=True)
                    nc.vector.tensor_copy(
                        relh[0:48, h * S + g * 480: h * S + g * 480 + cnt * 48],
                        ps[0:48, 0:cnt * 48])
                    wsrc = bass.AP(tensor=ps.tensor,
                                   offset=ps[64:112, :].offset,
                                   ap=[ps[64:112, :].ap[0], [48, cnt], [1, 48]])
                    for Qx in (QaugA, QaugB):
                        wdst = bass.AP(tensor=Qx.tensor,
                                       offset=Qx[64:112, :].offset + h * S + g * 10,
                                       ap=[Qx[64:112, :].ap[0], [1, cnt], [48, 48]])
                        nc.vector.tensor_copy(wdst, wsrc)

        with tc.tile_pool(name=pfx + "pTp", bufs=4) as pTp, \
             tc.tile_pool(name=pfx + "lrp", bufs=2) as lrp, \
             tc.tile_pool(name=pfx + "ps_S", bufs=2, space="PSUM") as ps_S, \
             tc.tile_pool(name=pfx + "yw", bufs=2) as yw, \
             tc.tile_pool(name=pfx + "ps_O", bufs=1, space="PSUM") as ps_O:

            def pv_step(h, psOs, step):
                kt, qt, pT = step
                q0, qw = QT[qt]
                vsl = slice(kt * VST + h * (HD + 1),
                            kt * VST + (h + 1) * (HD + 1))
                nc.tensor.matmul(psOs[qt][:, :qw], v[:, vsl], pT[:, :qw],
                                 start=(kt == 0), stop=(kt == KT - 1))

            def attn_pass(h, qts, psOs, project):
                """k-tile-outer attention over q-tiles `qts`: one 120-row
                matmul per (kt, qt) yields QK^T + both rel biases; PV trails
                the logits by two steps so each exp overlaps PE work. Then
                streamed normalization, and (if `project`) the output
                projection for the finished token tiles."""
                fill_window(h, 0)
                fill_window(h, 1)
                trail = []
                for kt in range(KT):
                    p = kt // 2
                    if kt >= 2 and kt % 2 == 0 and p + 1 <= (KT - 1) // 2:
                        fill_window(h, p + 1)
                    Qx = QaugA if p % 2 == 0 else QaugB
                    kc = slice(h * S + kt * 128, h * S + (kt + 1) * 128)
                    for qt in qts:
                        q0, qw = QT[qt]
                        psS = ps_S.tile([128, 512], F32, tag="s")
                        nc.tensor.matmul(
                            psS[:, :qw], Kaug[0:120, kc],
                            Qx[0:120, h * S + q0: h * S + q0 + qw],
                            start=True, stop=True)
                        pT = pTp.tile([128, 512], BF16, tag="p")
                        nc.scalar.activation(out=pT[:, :qw], in_=psS[:, :qw],
                                             func=ACTF.Exp)
                        trail.append((kt, qt, pT))
                        if len(trail) >= 3:
                            pv_step(h, psOs, trail.pop(0))
                for step in trail:
                    pv_step(h, psOs, step)

                # normalize: outT = psO[0:64] * broadcast(1/l); the broadcast
                # matmul borrows a rotating ps_S buffer (no extra PSUM bank).
                # One-step software pipeline so PE and DVE stream.
                prev = None
                psBs = {}

                def norm_tail(qt):
                    q0, qw = QT[qt]
                    lrb = lrp.tile([HD, 512], BF16, tag="lrb")
                    nc.vector.tensor_copy(lrb[:, :qw], psBs[qt][0:HD, :qw])
                    nc.vector.tensor_mul(outT[h][:, q0:q0 + qw],
                                         psOs[qt][0:HD, :qw], lrb[:, :qw])

                for qt in qts:
                    q0, qw = QT[qt]
                    lr = lrp.tile([1, 512], BF16, tag="lr")
                    with nc.allow_low_precision(reason="1/l in bf16: uniform "
                                                "2^-9 noise, gate is 2e-2"):
                        nc.vector.reciprocal(out=lr[:, :qw],
                                             in_=psOs[qt][HD:HD + 1, :qw])
                    psB = ps_S.tile([128, 512], F32, tag="s")
                    nc.tensor.matmul(psB[0:HD, :qw], ones64, lr[0:1, :qw],
                                     start=True, stop=True)
                    psBs[qt] = psB
                    if prev is not None:
                        norm_tail(prev)
                    prev = qt
                norm_tail(prev)

                if project:
                    for qt in qts:
                        q0, qw = QT[qt]
                        for ts in range(q0 // 128, (q0 + qw) // 128):
                            psA = ps_S.tile([128, 512], F32, tag="s")
                            psB2 = ps_S.tile([128, 512], F32, tag="s")
                            for j in range(NHC):
                                srcj = outT[j][:, ts * 128:(ts + 1) * 128]
                                nc.tensor.matmul(psA, srcj, wp_sb[j][:, 0:512],
                                                 start=(j == 0),
                                                 stop=(j == NHC - 1))
                                nc.tensor.matmul(psB2[:, 0:256], srcj,
                                                 wp_sb[j][:, 512:768],
                                                 start=(j == 0),
                                                 stop=(j == NHC - 1))
                            y_sb = yw.tile([128, D], F32, tag="ysb")
                            nc.vector.tensor_copy(y_sb[:, 0:512], psA)
                            nc.vector.tensor_copy(y_sb[:, 512:768],
                                                  psB2[:, 0:256])
                            nc.sync.dma_start(
                                out=y[ts * 128:(ts + 1) * 128, :], in_=y_sb)

            for h in range(NHC):
                psOs = [ps_O.tile([HD + 1, 512], F32, name=f"psO{h}{qt}",
                                  tag=f"o{qt}")
                        for qt in range(NQT)]
                if h < NHC - 1:
                    attn_pass(h, list(range(NQT)), psOs, project=False)
                else:
                    # split the last head so the projection of the first
                    # token block overlaps the second block's attention
                    attn_pass(h, [0, 1, 2], psOs, project=True)
                    attn_pass(h, [3, 4], psOs, project=True)

def build_nc(num_devices=N_CORES, reps=1):
    nc = bacc.Bacc("TRN2", target_bir_lowering=False, debug=False,
                   num_devices=num_devices)
    aps = (
        nc.dram_tensor("xT", [D, S], BF16, kind="ExternalInput").ap(),
        nc.dram_tensor("wqk", [D, 2 * NHC * HD], BF16, kind="ExternalInput").ap(),
        nc.dram_tensor("bqk", [128, NHC], F32, kind="ExternalInput").ap(),
        nc.dram_tensor("wv", [D, NHC * HD], BF16, kind="ExternalInput").ap(),
        nc.dram_tensor("wp", [NHC, HD, D], BF16, kind="ExternalInput").ap(),
        nc.dram_tensor("RhT", [HD, S], BF16, kind="ExternalInput").ap(),
        nc.dram_tensor("RwT", [HD, S], BF16, kind="ExternalInput").ap(),
        nc.dram_tensor("Estat", [56, S], BF16, kind="ExternalInput").ap(),
        nc.dram_tensor("y", [S, D], F32, kind="ExternalOutput").ap(),
    )
    with tile.TileContext(nc) as tc:
        for rep in range(reps):
            _emit(tc, nc, aps, pfx=f"r{rep}_" if reps > 1 else "")
    nc.compile()
    return nc


def prep_core_inputs(c, x, qkv_w, qkv_b, proj_w, rel_pos_h, rel_pos_w):
    bf16 = mybir.dt.np(BF16)
    b = c // 4
    heads = [3 * (c % 4) + j for j in range(NHC)]
    f32 = np.float32
    xTa = np.ascontiguousarray(np.asarray(x, f32)[b].reshape(S, D).T).astype(bf16)
    qkv_w = np.asarray(qkv_w, f32)
    qkv_b = np.asarray(qkv_b, f32)
    wq = np.concatenate([qkv_w[:, h * HD:(h + 1) * HD] for h in heads], 1) * f32(SCALE)
    wk = np.concatenate([qkv_w[:, D + h * HD:D + (h + 1) * HD] for h in heads], 1)
    wqka = np.ascontiguousarray(np.concatenate([wq, wk], 1)).astype(bf16)
    bq = [qkv_b[h * HD:(h + 1) * HD] * f32(SCALE) for h in heads]
    bk = [qkv_b[D + h * HD:D + (h + 1) * HD] for h in heads]
    # per-M-tile half-stacked biases: [q0|q1], [q2|k0], [k1|k2]
    halves = [bq[0], bq[1], bq[2], bk[0], bk[1], bk[2]]
    bqka = np.stack([np.concatenate([halves[2 * m], halves[2 * m + 1]])
                     for m in range(NHC)], 1).astype(f32)
    wva = np.ascontiguousarray(
        np.concatenate([qkv_w[:, 2 * D + h * HD:2 * D + (h + 1) * HD]
                        for h in heads], 1)).astype(bf16)
    wpa = np.ascontiguousarray(
        np.stack([np.asarray(proj_w, f32)[h * HD:(h + 1) * HD, :]
                  for h in heads], 0)).astype(bf16)
    coords = np.arange(H)[:, None] - np.arange(H)[None, :] + (H - 1)
    Rh = np.asarray(rel_pos_h, f32)[coords]      # [hq, hk, c]
    Rw = np.asarray(rel_pos_w, f32)[coords]      # [wq, wk, c]
    # The reference builds the rel bias from the UNSCALED q; we fold `SCALE`
    # into wq/bq, so fold the exact inverse (8.0) into the rel tables.
    inv = f32(1.0 / SCALE)
    RhTa = (np.ascontiguousarray(np.transpose(Rh, (2, 0, 1)).reshape(HD, S))
            * inv).astype(bf16)
    RwTa = (np.ascontiguousarray(np.transpose(Rw, (2, 0, 1)).reshape(HD, S))
            * inv).astype(bf16)
    # static one-hots for the augmented-K logits matmul: rows 0-47 rel_w
    # (k%48), rows 48-55 rel_h window selector (k//48 - pair base)
    E = np.zeros((56, S), bf16)
    kk = np.arange(S)
    E[kk % W, kk] = 1.0
    jj = kk // W - (256 * (kk // 256)) // W
    E[48 + jj, kk] = 1.0
    return {"xT": xTa, "wqk": wqka, "bqk": bqka, "wv": wva, "wp": wpa,
            "RhT": RhTa, "RwT": RwTa, "Estat": E}


_NC_CACHE = {}


def _get_nc(**kw):
    key = str(sorted(kw.items()))
    if key not in _NC_CACHE:
        _NC_CACHE[key] = build_nc(**kw)
    return _NC_CACHE[key]


def gather_output(ys, qkv_b, proj_w, proj_b):
    f32 = np.float32
    bp_eff = (np.asarray(proj_b, f32)
              + np.asarray(qkv_b, f32)[2 * D:] @ np.asarray(proj_w, f32))
    out = np.empty((B, H, W, D), f32)
    for b in range(B):
        acc = ys[4 * b].copy()
        for j in range(1, 4):
            acc += ys[4 * b + j]
        acc += bp_eff
        out[b] = acc.reshape(H, W, D)
    return out


def kernel(x, qkv_w, qkv_b, proj_w, proj_b, rel_pos_h, rel_pos_w):
    from concourse.bass_utils import run_bass_kernel_spmd
    nc = _get_nc()
    in_maps = [prep_core_inputs(c, x, qkv_w, qkv_b, proj_w, rel_pos_h, rel_pos_w)
               for c in range(N_CORES)]
    res = run_bass_kernel_spmd(nc, in_maps, core_ids=list(range(N_CORES)))
    ys = [res.results[c]["y"] for c in range(N_CORES)]
    return gather_output(ys, qkv_b, proj_w, proj_b)


# revision 29
# speedup vs baseline: 7313.2546x; 1.2456x over previous
"""Trainium2 Bass kernel for nn_Attention_15857019256917 (ViTDet-style attention
with decomposed relative position bias).

Sharding: data-parallel over B (2) x head-parallel (12 heads -> 4 groups of 3)
= 8 cores. Each core computes 3 heads of attention for one batch element plus
its partial output projection (rows of proj_w for its heads); the host sums the
4 partials per batch element (row-parallel linear unshard) and adds the bias
(with the v-bias folded in exactly: P@(V+1 bv^T)/l = PV/l + bv^T).

bf16 datapath (rel-err gate is 2e-2; measured ~5e-3): all matmul operands are
bf16 (1 PE cycle/col vs fp32's 4), PSUM accumulation stays fp32.

The logits for one k-tile need a single 120-row matmul: the stationary
operand stacks k (rows 0-63) with one-hot rows selecting rel_w (64-111,
k%48) and a windowed rel_h (112-119, k//48 - pair base); the moving operand
stacks q with the per-head rel_w / rel_h-window tables. Two Qaug variants
alternate per k-tile pair so the next pair's window DMA never touches rows
the in-flight matmuls read. The PV matmul trails the logits by two steps so
the softmax exp (ACT is the throughput floor at ~154us/rep) always overlaps
PE work.

Engine assignment: PE matmuls; ACT exp only; DVE psum->sbuf copies, bias
adds, reciprocal, normalization multiply; SWDGE(Pool-issued) side DMAs
(tables, one-hots, rel_h windows); SP queue x/weight loads, parked-half
shifts and y stores.

All tiles come from two pools that live across the whole NEFF (no per-phase
pool scopes): tensors written by rep r+1's prologue while rep r's attention
still reads them (Kaug/Qaug/relh/v) are double-buffered, so consecutive
kernel instances in a reps>1 NEFF pipeline across engines.
"""
import sys

sys.path.insert(0, "/opt/trn_rl_repo")

import numpy as np

import concourse.bass as bass
import concourse.bacc as bacc
import concourse.tile as tile
from concourse import mybir

F32 = mybir.dt.float32
BF16 = mybir.dt.bfloat16
ACTF = mybir.ActivationFunctionType

B, H, W, D = 2, 48, 48, 768
NH, HD = 12, 64
S = H * W                      # 2304
SCALE = HD ** -0.5
N_CORES = 8
NHC = 3                        # heads per core
KT = S // 128                  # 18 key tiles
TOKT = S // 128                # 18 token tiles
KCH = D // 128                 # 6 contraction chunks
QT = [(0, 512), (512, 512), (1024, 512), (1536, 512), (2048, 256)]
NQT = len(QT)
VST = NHC * (HD + 1)           # 195: per-ktile V layout [v_h0|1|v_h1|1|v_h2|1]


def _ap(t, off_elems, dims):
    """Raw AP on tile t: partition dim copied, free dims = [[step, count], ...]."""
    return bass.AP(tensor=t.tensor, offset=t.offset + off_elems, ap=[t.ap[0]] + dims)


def _alloc_rep_tiles(sb):
    """Per-rep rotating SBUF tiles. bufs=2 on everything rep r+1's prologue
    writes while rep r's attention still reads."""
    T = {}
    T["Kaug"] = sb.tile([128, NHC * S], BF16, name="Kaug", tag="Kaug", bufs=2)
    T["QaugA"] = sb.tile([128, NHC * S], BF16, name="QaugA", tag="QaugA", bufs=2)
    T["QaugB"] = sb.tile([128, NHC * S], BF16, name="QaugB", tag="QaugB", bufs=2)
    T["relh"] = sb.tile([48, NHC * S], BF16, name="relh", tag="relh", bufs=2)
    T["outT"] = [sb.tile([HD, S], BF16, name=f"outT{j}", tag=f"outT{j}",
                         bufs=2)
                 for j in range(NHC)]
    T["v"] = sb.tile([128, TOKT * VST], BF16, name="v", tag="v", bufs=2)
    T["xs"] = [sb.tile([128, S], BF16, name=f"x{k}", tag=f"x{k}")
               for k in range(KCH)]
    T["wqk_sb"] = sb.tile([128, KCH * 2 * NHC * HD], BF16, name="wqk_sb",
                          tag="wqk_sb")
    T["wv_sb"] = sb.tile([128, KCH * NHC * HD], BF16, name="wv_sb",
                         tag="wv_sb")
    T["bqk_sb"] = sb.tile([128, NHC], F32, name="bqk_sb", tag="bqk_sb")
    return T


def _alloc_const_tiles(sb):
    """Constants shared by every rep: one buffer, loaded once."""
    C = {}
    C["RhT_sb"] = sb.tile([HD, S], BF16, name="RhT_sb", tag="RhT_sb")
    C["RwT_sb"] = sb.tile([HD, S], BF16, name="RwT_sb", tag="RwT_sb")
    C["wp_sb"] = [sb.tile([HD, D], BF16, name=f"wp{j}", tag=f"wp{j}")
                  for j in range(NHC)]
    C["ones64"] = sb.tile([1, HD], BF16, name="ones64", tag="ones64")
    return C


def _prologue_items(nc, aps, ps, T, C, first):
    """Emission closures for one rep's prologue (input loads, qkv
    projections, V, rel tables), to be interleaved into the previous rep's
    ACT-bound attention stream so this PE/DMA work hides under it."""
    xT, wqk, bqk, wv, wp, RhT, RwT, Estat, y = aps
    Kaug, QaugA, QaugB = T["Kaug"], T["QaugA"], T["QaugB"]
    relh, v, xs = T["relh"], T["v"], T["xs"]
    wqk_sb, wv_sb, bqk_sb = T["wqk_sb"], T["wv_sb"], T["bqk_sb"]
    items = []

    def loads():
        if first:
            # constants shared by every rep: load once (also avoids a
            # cross-rep WAR between these loads and the previous rep's
            # interleaved projection reads)
            nc.gpsimd.dma_start(out=C["RhT_sb"], in_=RhT)
            nc.gpsimd.dma_start(out=C["RwT_sb"], in_=RwT)
            for j in range(NHC):
                nc.gpsimd.dma_start(out=C["wp_sb"][j], in_=wp[j])
            nc.vector.memset(C["ones64"], 1.0)
        nc.sync.dma_start(out=bqk_sb, in_=bqk)
        nc.vector.memset(_ap(v, HD, [[VST, TOKT], [HD + 1, NHC]]), 1.0)
        for k in range(KCH):
            nc.sync.dma_start(out=xs[k], in_=xT[k * 128:(k + 1) * 128, :])
        for k in range(KCH):
            nc.sync.dma_start(out=wqk_sb[:, k * 384:(k + 1) * 384],
                              in_=wqk[k * 128:(k + 1) * 128, :])
            nc.sync.dma_start(out=wv_sb[:, k * 192:(k + 1) * 192],
                              in_=wv[k * 128:(k + 1) * 128, :])
        # static one-hot rows of Kaug (high halves park in QaugB, so no
        # ordering dependency on phase 1)
        for j in range(NHC):
            nc.gpsimd.dma_start(out=Kaug[64:112, j * S:(j + 1) * S],
                                in_=Estat[0:48, :])
            nc.gpsimd.dma_start(out=Kaug[112:120, j * S:(j + 1) * S],
                                in_=Estat[48:56, :])
    items.append(loads)

    # M-tiles (128 rows = two 64-channel halves): T0=[q0|q1] T1=[q2|k0]
    # T2=[k1|k2]. Low halves copy straight to rows 0-63 of their dest
    # tensor; high halves park in QaugB rows 64-127 (same column range),
    # then an intra-tensor DMA partition-shifts them to the dest.
    lo_dest = [(QaugA, 0), (QaugA, 2), (Kaug, 1)]
    hi_dest = [(QaugA, 1), (Kaug, 0), (Kaug, 2)]
    for m in range(NHC):
        for (n0, nw) in QT:
            def qk_group(m=m, n0=n0, nw=nw):
                psq = ps.tile([128, 512], F32, name="psq", tag="s", bufs=3)
                for k in range(KCH):
                    nc.tensor.matmul(
                        psq[:, :nw],
                        wqk_sb[:, k * 384 + m * 128: k * 384 + (m + 1) * 128],
                        xs[k][:, n0:n0 + nw],
                        start=(k == 0), stop=(k == KCH - 1))
                lt_, lh = lo_dest[m]
                ht_, hh = hi_dest[m]
                nc.vector.tensor_scalar_add(
                    lt_[0:64, lh * S + n0: lh * S + n0 + nw],
                    psq[0:64, :nw], bqk_sb[0:64, m:m + 1])
                nc.vector.tensor_scalar_add(
                    QaugB[64:128, hh * S + n0: hh * S + n0 + nw],
                    psq[64:128, :nw], bqk_sb[64:128, m:m + 1])
            items.append(qk_group)
    for ts in range(TOKT):
        def v_group(ts=ts):
            psv = ps.tile([128, 512], F32, name="psv", tag="s", bufs=3)
            for k in range(KCH):
                nc.tensor.matmul(
                    psv[:, 0:NHC * HD],
                    xs[k][:, ts * 128:(ts + 1) * 128],
                    wv_sb[:, k * 192:(k + 1) * 192],
                    start=(k == 0), stop=(k == KCH - 1))
            vdst = _ap(v, ts * VST, [[HD + 1, NHC], [1, HD]])
            vsrc = _ap(psv, 0, [[HD, NHC], [1, HD]])
            nc.vector.tensor_copy(vdst, vsrc)
        items.append(v_group)

    def shifts():
        for m in range(NHC):
            ht_, hh = hi_dest[m]
            nc.sync.dma_start(out=ht_[0:64, hh * S:(hh + 1) * S],
                              in_=QaugB[64:128, hh * S:(hh + 1) * S])
        # q lives in both Qaug variants
        nc.vector.tensor_copy(QaugB[0:64, :], QaugA[0:64, :])
    items.append(shifts)

    # rel tables (PE-only): rel_h lands at psum rows 0-47 -> relh staging
    # tile; rel_w at rows 64-111 -> both Qaug variants
    for h in range(NHC):
        for g in range(5):
            def rel_group(h=h, g=g):
                cnt = 10 if g < 4 else 8
                psr = ps.tile([128, 512], F32, name="psr", tag="s", bufs=3)
                for i in range(cnt):
                    r = g * 10 + i
                    nc.tensor.matmul(
                        psr[0:48, i * 48:(i + 1) * 48],
                        C["RhT_sb"][:, r * 48:(r + 1) * 48],
                        QaugA[0:64, h * S + r * 48: h * S + (r + 1) * 48],
                        start=(i == 0), stop=(i == cnt - 1))
                    # out at base partition 64 (col-tiled); the sim's
                    # zero-region bookkeeping mis-indexes partition-offset
                    # psum APs, so skip its group check (single writer)
                    nc.tensor.matmul(
                        psr[64:112, i * 48:(i + 1) * 48],
                        C["RwT_sb"][:, r * 48:(r + 1) * 48],
                        bass.AP(tensor=QaugA.tensor,
                                offset=QaugA.offset + h * S + r,
                                ap=[QaugA[0:64, :].ap[0], [48, 48]]),
                        start=(i == 0), stop=(i == cnt - 1),
                        skip_group_check=True)
                nc.vector.tensor_copy(
                    relh[0:48, h * S + g * 480: h * S + g * 480 + cnt * 48],
                    psr[0:48, 0:cnt * 48])
                wsrc = bass.AP(tensor=psr.tensor,
                               offset=psr[64:112, :].offset,
                               ap=[psr[64:112, :].ap[0], [48, cnt], [1, 48]])
                for Qx in (QaugA, QaugB):
                    wdst = bass.AP(
                        tensor=Qx.tensor,
                        offset=Qx[64:112, :].offset + h * S + g * 10,
                        ap=[Qx[64:112, :].ap[0], [1, cnt], [48, 48]])
                    nc.vector.tensor_copy(wdst, wsrc)
            items.append(rel_group)
    return items


def _proj_items(nc, aps, ps, sb, T, C):
    """Output-projection closures for a finished rep: interleaved into the
    next rep's attention (or emitted directly for the last rep). Split per
    psum tile so at most one extra "s" buffer is held at a time."""
    y = aps[8]
    items = []
    for ts in range(TOKT):
        def proj_a(ts=ts):
            psA = ps.tile([128, 512], F32, name="psA", tag="s", bufs=3)
            for j in range(NHC):
                nc.tensor.matmul(psA,
                                 T["outT"][j][:, ts * 128:(ts + 1) * 128],
                                 C["wp_sb"][j][:, 0:512],
                                 start=(j == 0), stop=(j == NHC - 1))
            y_sb = sb.tile([128, D], F32, name="y_sb", tag="ysb", bufs=1)
            nc.vector.tensor_copy(y_sb[:, 0:512], psA)
            T["_ysb"] = y_sb
        def proj_b(ts=ts):
            psB2 = ps.tile([128, 512], F32, name="psB2", tag="s", bufs=3)
            for j in range(NHC):
                nc.tensor.matmul(psB2[:, 0:256],
                                 T["outT"][j][:, ts * 128:(ts + 1) * 128],
                                 C["wp_sb"][j][:, 512:768],
                                 start=(j == 0), stop=(j == NHC - 1))
            y_sb = T["_ysb"]
            nc.vector.tensor_copy(y_sb[:, 512:768], psB2[:, 0:256])
            nc.sync.dma_start(out=y[ts * 128:(ts + 1) * 128, :], in_=y_sb)
        items.append(proj_a)
        items.append(proj_b)
    return items


def _attention(nc, ps, sb, T, C, interleave):
    """k-tile-outer attention for all heads. One 120-row matmul per
    (kt, qt) yields QK^T + both rel biases; PV trails the logits by two
    steps so the exp always overlaps PE work. `interleave` closures (next
    rep's prologue, previous rep's projection) are popped between steps to
    fill PE's idle time under the ACT-bound exp stream."""
    Kaug, QaugA, QaugB, relh, v = (T["Kaug"], T["QaugA"], T["QaugB"],
                                   T["relh"], T["v"])
    queue = list(interleave)
    nsteps = NHC * KT * NQT
    stride = max(1, nsteps // (len(queue) + 1)) if queue else nsteps
    step_no = 0

    def fill_window(h, p):
        Qx = QaugA if p % 2 == 0 else QaugB
        base = (256 * p) // 48
        n = min(8, 48 - base)
        nc.gpsimd.dma_start(out=Qx[112:112 + n, h * S:(h + 1) * S],
                            in_=relh[base:base + n, h * S:(h + 1) * S])

    def pv_step(h, psOs, step):
        kt, qt, pT = step
        q0, qw = QT[qt]
        vsl = slice(kt * VST + h * (HD + 1), kt * VST + (h + 1) * (HD + 1))
        nc.tensor.matmul(psOs[qt][:, :qw], v[:, vsl], pT[:, :qw],
                         start=(kt == 0), stop=(kt == KT - 1))

    for h in range(NHC):
        psOs = [ps.tile([HD + 1, 512], F32, name=f"psO{h}{qt}", tag=f"o{qt}")
                for qt in range(NQT)]
        fill_window(h, 0)
        fill_window(h, 1)
        trail = []
        for kt in range(KT):
            p = kt // 2
            if kt >= 2 and kt % 2 == 0 and p + 1 <= (KT - 1) // 2:
                fill_window(h, p + 1)
            Qx = QaugA if p % 2 == 0 else QaugB
            kc = slice(h * S + kt * 128, h * S + (kt + 1) * 128)
            for qt, (q0, qw) in enumerate(QT):
                if queue and step_no % stride == 0:
                    queue.pop(0)()
                step_no += 1
                psS = ps.tile([128, 512], F32, name="psS", tag="s", bufs=3)
                nc.tensor.matmul(
                    psS[:, :qw], Kaug[0:120, kc],
                    Qx[0:120, h * S + q0: h * S + q0 + qw],
                    start=True, stop=True)
                pT = sb.tile([128, 512], BF16, name="pT", tag="p", bufs=3)
                nc.scalar.activation(out=pT[:, :qw], in_=psS[:, :qw],
                                     func=ACTF.Exp)
                trail.append((kt, qt, pT))
                if len(trail) >= 3:
                    pv_step(h, psOs, trail.pop(0))
        for step in trail:
            pv_step(h, psOs, step)

        # normalize: outT = psO[0:64] * broadcast(1/l); the broadcast matmul
        # borrows a rotating "s" psum buffer. One-step software pipeline so
        # PE and DVE stream instead of ping-ponging.
        psBs = {}

        def norm_tail(qt):
            q0, qw = QT[qt]
            lrb = sb.tile([HD, 512], BF16, name="lrb", tag="lrb", bufs=2)
            nc.vector.tensor_copy(lrb[:, :qw], psBs[qt][0:HD, :qw])
            nc.vector.tensor_mul(T["outT"][h][:, q0:q0 + qw],
                                 psOs[qt][0:HD, :qw], lrb[:, :qw])

        prev = None
        for qt, (q0, qw) in enumerate(QT):
            lr = sb.tile([1, 512], BF16, name="lr", tag="lr", bufs=2)
            with nc.allow_low_precision(reason="1/l in bf16: uniform 2^-9 "
                                        "noise, gate is 2e-2"):
                nc.vector.reciprocal(out=lr[:, :qw],
                                     in_=psOs[qt][HD:HD + 1, :qw])
            psB = ps.tile([128, 512], F32, name="psB", tag="s", bufs=3)
            nc.tensor.matmul(psB[0:HD, :qw], C["ones64"], lr[0:1, :qw],
                             start=True, stop=True)
            psBs[qt] = psB
            if prev is not None:
                norm_tail(prev)
            prev = qt
        norm_tail(prev)

    # leftover interleave items (shouldn't normally remain)
    for it in queue:
        it()


def build_nc(num_devices=N_CORES, reps=1):
    from contextlib import ExitStack
    nc = bacc.Bacc("TRN2", target_bir_lowering=False, debug=False,
                   num_devices=num_devices)
    aps = (
        nc.dram_tensor("xT", [D, S], BF16, kind="ExternalInput").ap(),
        nc.dram_tensor("wqk", [D, 2 * NHC * HD], BF16, kind="ExternalInput").ap(),
        nc.dram_tensor("bqk", [128, NHC], F32, kind="ExternalInput").ap(),
        nc.dram_tensor("wv", [D, NHC * HD], BF16, kind="ExternalInput").ap(),
        nc.dram_tensor("wp", [NHC, HD, D], BF16, kind="ExternalInput").ap(),
        nc.dram_tensor("RhT", [HD, S], BF16, kind="ExternalInput").ap(),
        nc.dram_tensor("RwT", [HD, S], BF16, kind="ExternalInput").ap(),
        nc.dram_tensor("Estat", [56, S], BF16, kind="ExternalInput").ap(),
        nc.dram_tensor("y", [S, D], F32, kind="ExternalOutput").ap(),
    )
    with tile.TileContext(nc) as tc:
        with ExitStack() as es:
            sb = es.enter_context(tc.tile_pool(name="sb", bufs=1))
            ps = es.enter_context(tc.tile_pool(name="ps", bufs=1,
                                               space="PSUM"))
            # software-pipelined across reps: rep r's attention emission
            # interleaves rep r+1's prologue and rep r-1's projection
            C = _alloc_const_tiles(sb)
            T_cur = _alloc_rep_tiles(sb)
            for it in _prologue_items(nc, aps, ps, T_cur, C, first=True):
                it()
            proj_prev = []
            for r in range(reps):
                nxt = []
                T_nxt = None
                if r + 1 < reps:
                    T_nxt = _alloc_rep_tiles(sb)
                    nxt = _prologue_items(nc, aps, ps, T_nxt, C, first=False)
                _attention(nc, ps, sb, T_cur, C, proj_prev + nxt)
                proj_prev = _proj_items(nc, aps, ps, sb, T_cur, C)
                if r == reps - 1:
                    for it in proj_prev:
                        it()
                T_cur = T_nxt
    nc.compile()
    return nc


def prep_core_inputs(c, x, qkv_w, qkv_b, proj_w, rel_pos_h, rel_pos_w):
    bf16 = mybir.dt.np(BF16)
    b = c // 4
    heads = [3 * (c % 4) + j for j in range(NHC)]
    f32 = np.float32
    xTa = np.ascontiguousarray(np.asarray(x, f32)[b].reshape(S, D).T).astype(bf16)
    qkv_w = np.asarray(qkv_w, f32)
    qkv_b = np.asarray(qkv_b, f32)
    wq = np.concatenate([qkv_w[:, h * HD:(h + 1) * HD] for h in heads], 1) * f32(SCALE)
    wk = np.concatenate([qkv_w[:, D + h * HD:D + (h + 1) * HD] for h in heads], 1)
    wqka = np.ascontiguousarray(np.concatenate([wq, wk], 1)).astype(bf16)
    bq = [qkv_b[h * HD:(h + 1) * HD] * f32(SCALE) for h in heads]
    bk = [qkv_b[D + h * HD:D + (h + 1) * HD] for h in heads]
    # per-M-tile half-stacked biases: [q0|q1], [q2|k0], [k1|k2]
    halves = [bq[0], bq[1], bq[2], bk[0], bk[1], bk[2]]
    bqka = np.stack([np.concatenate([halves[2 * m], halves[2 * m + 1]])
                     for m in range(NHC)], 1).astype(f32)
    wva = np.ascontiguousarray(
        np.concatenate([qkv_w[:, 2 * D + h * HD:2 * D + (h + 1) * HD]
                        for h in heads], 1)).astype(bf16)
    wpa = np.ascontiguousarray(
        np.stack([np.asarray(proj_w, f32)[h * HD:(h + 1) * HD, :]
                  for h in heads], 0)).astype(bf16)
    coords = np.arange(H)[:, None] - np.arange(H)[None, :] + (H - 1)
    Rh = np.asarray(rel_pos_h, f32)[coords]      # [hq, hk, c]
    Rw = np.asarray(rel_pos_w, f32)[coords]      # [wq, wk, c]
    # The reference builds the rel bias from the UNSCALED q; we fold `SCALE`
    # into wq/bq, so fold the exact inverse (8.0) into the rel tables.
    inv = f32(1.0 / SCALE)
    RhTa = (np.ascontiguousarray(np.transpose(Rh, (2, 0, 1)).reshape(HD, S))
            * inv).astype(bf16)
    RwTa = (np.ascontiguousarray(np.transpose(Rw, (2, 0, 1)).reshape(HD, S))
            * inv).astype(bf16)
    # static one-hots for the augmented-K logits matmul: rows 0-47 rel_w
    # (k%48), rows 48-55 rel_h window selector (k//48 - pair base)
    E = np.zeros((56, S), bf16)
    kk = np.arange(S)
    E[kk % W, kk] = 1.0
    jj = kk // W - (256 * (kk // 256)) // W
    E[48 + jj, kk] = 1.0
    return {"xT": xTa, "wqk": wqka, "bqk": bqka, "wv": wva, "wp": wpa,
            "RhT": RhTa, "RwT": RwTa, "Estat": E}


_NC_CACHE = {}


def _get_nc(**kw):
    key = str(sorted(kw.items()))
    if key not in _NC_CACHE:
        _NC_CACHE[key] = build_nc(**kw)
    return _NC_CACHE[key]


def gather_output(ys, qkv_b, proj_w, proj_b):
    f32 = np.float32
    bp_eff = (np.asarray(proj_b, f32)
              + np.asarray(qkv_b, f32)[2 * D:] @ np.asarray(proj_w, f32))
    out = np.empty((B, H, W, D), f32)
    for b in range(B):
        acc = ys[4 * b].copy()
        for j in range(1, 4):
            acc += ys[4 * b + j]
        acc += bp_eff
        out[b] = acc.reshape(H, W, D)
    return out


def kernel(x, qkv_w, qkv_b, proj_w, proj_b, rel_pos_h, rel_pos_w):
    from concourse.bass_utils import run_bass_kernel_spmd
    nc = _get_nc()
    in_maps = [prep_core_inputs(c, x, qkv_w, qkv_b, proj_w, rel_pos_h, rel_pos_w)
               for c in range(N_CORES)]
    res = run_bass_kernel_spmd(nc, in_maps, core_ids=list(range(N_CORES)))
    ys = [res.results[c]["y"] for c in range(N_CORES)]
    return gather_output(ys, qkv_b, proj_w, proj_b)


# revision 35
# speedup vs baseline: 7996.1142x; 1.0934x over previous
"""Trainium2 Bass kernel for nn_Attention_15857019256917 (ViTDet-style attention
with decomposed relative position bias).

Sharding: data-parallel over B (2) x head-parallel (12 heads -> 4 groups of 3)
= 8 cores. Each core computes 3 heads of attention for one batch element plus
its partial output projection (rows of proj_w for its heads); the host sums the
4 partials per batch element (row-parallel linear unshard) and adds the bias
(with the v-bias folded in exactly: P@(V+1 bv^T)/l = PV/l + bv^T).

bf16 datapath (rel-err gate is 2e-2; measured ~5e-3): all matmul operands are
bf16 (1 PE cycle/col vs fp32's 4), PSUM accumulation stays fp32.

The logits for one k-tile take a single 120-row matmul: the stationary
operand stacks k (rows 0-63) with one-hot rows selecting rel_w (64-111,
k%48) and a windowed rel_h (112-119, k//48 - pair base); the moving operand
stacks q with the per-head rel_w / rel_h-window tables. Two Qaug variants
alternate per k-tile pair so the next pair's window DMA never touches rows
the in-flight matmuls read ([0:120] excludes the spare rows). The PV matmul
trails the logits by three steps so the softmax exp (ACT is the throughput
floor) always overlaps PE work.

Engine assignment: PE all matmuls (incl. the 1/l broadcast); ACT exp only;
DVE psum->sbuf copies, qkv bias adds, reciprocal, normalization multiply;
SWDGE (Pool-issued) side DMAs (rel tables, one-hots, rel_h windows); SP
HWDGE queue x/weight loads, parked-half shifts and y stores.

All tiles come from two pools that live across the whole NEFF (no per-phase
pool scopes, so no all-engine barriers between phases). In a reps>1 NEFF the
emission itself is software-pipelined across kernel instances: rep r's
ACT-bound attention stream interleaves rep r+1's prologue (input loads, qkv
projection, V, rel tables) and rep r-1's output projection as closures
popped between attention steps; tensors written by rep r+1's prologue while
rep r still reads them (Kaug/Qaug/relh/v/outT) are double-buffered. Engines
execute their queues in order, so this emission interleave is what converts
per-rep latency into pipelined throughput.
"""
import sys

sys.path.insert(0, "/opt/trn_rl_repo")

import numpy as np

import concourse.bass as bass
import concourse.bacc as bacc
import concourse.tile as tile
from concourse import mybir

F32 = mybir.dt.float32
BF16 = mybir.dt.bfloat16
ACTF = mybir.ActivationFunctionType

B, H, W, D = 2, 48, 48, 768
NH, HD = 12, 64
S = H * W                      # 2304
SCALE = HD ** -0.5
N_CORES = 8
NHC = 3                        # heads per core
KT = S // 128                  # 18 key tiles
TOKT = S // 128                # 18 token tiles
KCH = D // 128                 # 6 contraction chunks
QT = [(0, 512), (512, 512), (1024, 512), (1536, 512), (2048, 256)]
NQT = len(QT)
VST = NHC * (HD + 1)           # 195: per-ktile V layout [v_h0|1|v_h1|1|v_h2|1]


def _ap(t, off_elems, dims):
    """Raw AP on tile t: partition dim copied, free dims = [[step, count], ...]."""
    return bass.AP(tensor=t.tensor, offset=t.offset + off_elems, ap=[t.ap[0]] + dims)


def _alloc_rep_tiles(sb):
    """Per-rep rotating SBUF tiles. bufs=2 on everything rep r+1's prologue
    writes while rep r's attention still reads."""
    T = {}
    T["Kaug"] = sb.tile([128, NHC * S], BF16, name="Kaug", tag="Kaug", bufs=2)
    T["QaugA"] = sb.tile([128, NHC * S], BF16, name="QaugA", tag="QaugA", bufs=2)
    T["QaugB"] = sb.tile([128, NHC * S], BF16, name="QaugB", tag="QaugB", bufs=2)
    T["relh"] = sb.tile([48, NHC * S], BF16, name="relh", tag="relh", bufs=2)
    T["outT"] = [sb.tile([HD, S], BF16, name=f"outT{j}", tag=f"outT{j}",
                         bufs=2)
                 for j in range(NHC)]
    T["v"] = sb.tile([128, TOKT * VST], BF16, name="v", tag="v", bufs=2)
    T["xs"] = [sb.tile([128, S], BF16, name=f"x{k}", tag=f"x{k}")
               for k in range(KCH)]
    T["wqk_sb"] = sb.tile([128, KCH * 2 * NHC * HD], BF16, name="wqk_sb",
                          tag="wqk_sb")
    T["wv_sb"] = sb.tile([128, KCH * NHC * HD], BF16, name="wv_sb",
                         tag="wv_sb")
    T["bqk_sb"] = sb.tile([128, NHC], F32, name="bqk_sb", tag="bqk_sb")
    return T


def _alloc_const_tiles(sb):
    """Constants shared by every rep: one buffer, loaded once."""
    C = {}
    C["RhT_sb"] = sb.tile([HD, S], BF16, name="RhT_sb", tag="RhT_sb")
    C["RwT_sb"] = sb.tile([HD, S], BF16, name="RwT_sb", tag="RwT_sb")
    C["wp_sb"] = [sb.tile([HD, D], BF16, name=f"wp{j}", tag=f"wp{j}")
                  for j in range(NHC)]
    C["ones64"] = sb.tile([1, HD], BF16, name="ones64", tag="ones64")
    return C


def _prologue_items(nc, aps, ps, T, C, first):
    """Emission closures for one rep's prologue (input loads, qkv
    projections, V, rel tables), to be interleaved into the previous rep's
    ACT-bound attention stream so this PE/DMA work hides under it."""
    xT, wqk, bqk, wv, wp, RhT, RwT, Estat, y = aps
    Kaug, QaugA, QaugB = T["Kaug"], T["QaugA"], T["QaugB"]
    relh, v, xs = T["relh"], T["v"], T["xs"]
    wqk_sb, wv_sb, bqk_sb = T["wqk_sb"], T["wv_sb"], T["bqk_sb"]
    items = []

    def loads():
        if first:
            # constants shared by every rep: load once (also avoids a
            # cross-rep WAR between these loads and the previous rep's
            # interleaved projection reads)
            nc.gpsimd.dma_start(out=C["RhT_sb"], in_=RhT)
            nc.gpsimd.dma_start(out=C["RwT_sb"], in_=RwT)
            for j in range(NHC):
                nc.gpsimd.dma_start(out=C["wp_sb"][j], in_=wp[j])
            nc.vector.memset(C["ones64"], 1.0)
        nc.sync.dma_start(out=bqk_sb, in_=bqk)
        nc.vector.memset(_ap(v, HD, [[VST, TOKT], [HD + 1, NHC]]), 1.0)
        for k in range(KCH):
            nc.sync.dma_start(out=xs[k], in_=xT[k * 128:(k + 1) * 128, :])
        for k in range(KCH):
            nc.sync.dma_start(out=wqk_sb[:, k * 384:(k + 1) * 384],
                              in_=wqk[k * 128:(k + 1) * 128, :])
            nc.sync.dma_start(out=wv_sb[:, k * 192:(k + 1) * 192],
                              in_=wv[k * 128:(k + 1) * 128, :])
        # static one-hot rows of Kaug (high halves park in QaugB, so no
        # ordering dependency on phase 1)
        for j in range(NHC):
            nc.gpsimd.dma_start(out=Kaug[64:112, j * S:(j + 1) * S],
                                in_=Estat[0:48, :])
            nc.gpsimd.dma_start(out=Kaug[112:120, j * S:(j + 1) * S],
                                in_=Estat[48:56, :])
    items.append(loads)

    # M-tiles (128 rows = two 64-channel halves): T0=[q0|q1] T1=[q2|k0]
    # T2=[k1|k2]. Low halves copy straight to rows 0-63 of their dest
    # tensor; high halves park in QaugB rows 64-127 (same column range),
    # then an intra-tensor DMA partition-shifts them to the dest.
    lo_dest = [(QaugA, 0), (QaugA, 2), (Kaug, 1)]
    hi_dest = [(QaugA, 1), (Kaug, 0), (Kaug, 2)]
    for m in range(NHC):
        for (n0, nw) in QT:
            def qk_group(m=m, n0=n0, nw=nw):
                psq = ps.tile([128, 512], F32, name="psq", tag="s", bufs=3)
                for k in range(KCH):
                    nc.tensor.matmul(
                        psq[:, :nw],
                        wqk_sb[:, k * 384 + m * 128: k * 384 + (m + 1) * 128],
                        xs[k][:, n0:n0 + nw],
                        start=(k == 0), stop=(k == KCH - 1))
                lt_, lh = lo_dest[m]
                ht_, hh = hi_dest[m]
                nc.vector.tensor_scalar_add(
                    lt_[0:64, lh * S + n0: lh * S + n0 + nw],
                    psq[0:64, :nw], bqk_sb[0:64, m:m + 1])
                nc.vector.tensor_scalar_add(
                    QaugB[64:128, hh * S + n0: hh * S + n0 + nw],
                    psq[64:128, :nw], bqk_sb[64:128, m:m + 1])
            items.append(qk_group)
    for ts in range(TOKT):
        def v_group(ts=ts):
            psv = ps.tile([128, 512], F32, name="psv", tag="s", bufs=3)
            for k in range(KCH):
                nc.tensor.matmul(
                    psv[:, 0:NHC * HD],
                    xs[k][:, ts * 128:(ts + 1) * 128],
                    wv_sb[:, k * 192:(k + 1) * 192],
                    start=(k == 0), stop=(k == KCH - 1))
            vdst = _ap(v, ts * VST, [[HD + 1, NHC], [1, HD]])
            vsrc = _ap(psv, 0, [[HD, NHC], [1, HD]])
            nc.vector.tensor_copy(vdst, vsrc)
        items.append(v_group)

    def shifts():
        for m in range(NHC):
            ht_, hh = hi_dest[m]
            nc.sync.dma_start(out=ht_[0:64, hh * S:(hh + 1) * S],
                              in_=QaugB[64:128, hh * S:(hh + 1) * S])
        # q lives in both Qaug variants
        nc.vector.tensor_copy(QaugB[0:64, :], QaugA[0:64, :])
    items.append(shifts)

    # rel tables (PE-only): rel_h lands at psum rows 0-47 -> relh staging
    # tile; rel_w at rows 64-111 -> both Qaug variants
    for h in range(NHC):
        for g in range(5):
            def rel_group(h=h, g=g):
                cnt = 10 if g < 4 else 8
                psr = ps.tile([128, 512], F32, name="psr", tag="s", bufs=3)
                for i in range(cnt):
                    r = g * 10 + i
                    nc.tensor.matmul(
                        psr[0:48, i * 48:(i + 1) * 48],
                        C["RhT_sb"][:, r * 48:(r + 1) * 48],
                        QaugA[0:64, h * S + r * 48: h * S + (r + 1) * 48],
                        start=(i == 0), stop=(i == cnt - 1))
                    # out at base partition 64 (col-tiled); the sim's
                    # zero-region bookkeeping mis-indexes partition-offset
                    # psum APs, so skip its group check (single writer)
                    nc.tensor.matmul(
                        psr[64:112, i * 48:(i + 1) * 48],
                        C["RwT_sb"][:, r * 48:(r + 1) * 48],
                        bass.AP(tensor=QaugA.tensor,
                                offset=QaugA.offset + h * S + r,
                                ap=[QaugA[0:64, :].ap[0], [48, 48]]),
                        start=(i == 0), stop=(i == cnt - 1),
                        skip_group_check=True)
                nc.vector.tensor_copy(
                    relh[0:48, h * S + g * 480: h * S + g * 480 + cnt * 48],
                    psr[0:48, 0:cnt * 48])
                wsrc = bass.AP(tensor=psr.tensor,
                               offset=psr[64:112, :].offset,
                               ap=[psr[64:112, :].ap[0], [48, cnt], [1, 48]])
                for Qx in (QaugA, QaugB):
                    wdst = bass.AP(
                        tensor=Qx.tensor,
                        offset=Qx[64:112, :].offset + h * S + g * 10,
                        ap=[Qx[64:112, :].ap[0], [1, cnt], [48, 48]])
                    nc.vector.tensor_copy(wdst, wsrc)
            items.append(rel_group)
    return items


def _proj_items(nc, aps, ps, sb, T, C):
    """Output-projection closures for a finished rep: interleaved into the
    next rep's attention (or emitted directly for the last rep). Split per
    psum tile so at most one extra "s" buffer is held at a time."""
    y = aps[8]
    items = []
    for ts in range(TOKT):
        def proj_a(ts=ts):
            psA = ps.tile([128, 512], F32, name="psA", tag="s", bufs=3)
            for j in range(NHC):
                nc.tensor.matmul(psA,
                                 T["outT"][j][:, ts * 128:(ts + 1) * 128],
                                 C["wp_sb"][j][:, 0:512],
                                 start=(j == 0), stop=(j == NHC - 1))
            y_sb = sb.tile([128, D], F32, name="y_sb", tag="ysb", bufs=1)
            nc.vector.tensor_copy(y_sb[:, 0:512], psA)
            T["_ysb"] = y_sb
        def proj_b(ts=ts):
            psB2 = ps.tile([128, 512], F32, name="psB2", tag="s", bufs=3)
            for j in range(NHC):
                nc.tensor.matmul(psB2[:, 0:256],
                                 T["outT"][j][:, ts * 128:(ts + 1) * 128],
                                 C["wp_sb"][j][:, 512:768],
                                 start=(j == 0), stop=(j == NHC - 1))
            y_sb = T["_ysb"]
            nc.vector.tensor_copy(y_sb[:, 512:768], psB2[:, 0:256])
            nc.sync.dma_start(out=y[ts * 128:(ts + 1) * 128, :], in_=y_sb)
        items.append(proj_a)
        items.append(proj_b)
    return items


def _attention(nc, ps, sb, T, C, interleave, no_fills=False):
    """k-tile-outer attention for all heads. One 120-row matmul per
    (kt, qt) yields QK^T + both rel biases; PV trails the logits by two
    steps so the exp always overlaps PE work. `interleave` closures (next
    rep's prologue, previous rep's projection) are popped between steps to
    fill PE's idle time under the ACT-bound exp stream."""
    Kaug, QaugA, QaugB, relh, v = (T["Kaug"], T["QaugA"], T["QaugB"],
                                   T["relh"], T["v"])
    queue = list(interleave)
    nsteps = NHC * KT * NQT
    stride = max(1, nsteps // (len(queue) + 1)) if queue else nsteps
    step_no = 0

    def fill_window(h, p):
        if no_fills:
            return
        Qx = QaugA if p % 2 == 0 else QaugB
        base = (256 * p) // 48
        n = min(8, 48 - base)
        nc.gpsimd.dma_start(out=Qx[112:112 + n, h * S:(h + 1) * S],
                            in_=relh[base:base + n, h * S:(h + 1) * S])

    def pv_step(h, psOs, step):
        kt, qt, pT = step
        q0, qw = QT[qt]
        vsl = slice(kt * VST + h * (HD + 1), kt * VST + (h + 1) * (HD + 1))
        nc.tensor.matmul(psOs[qt][:, :qw], v[:, vsl], pT[:, :qw],
                         start=(kt == 0), stop=(kt == KT - 1))

    for h in range(NHC):
        psOs = [ps.tile([HD + 1, 512], F32, name=f"psO{h}{qt}", tag=f"o{qt}")
                for qt in range(NQT)]
        fill_window(h, 0)
        fill_window(h, 1)
        trail = []
        for kt in range(KT):
            p = kt // 2
            if kt >= 2 and kt % 2 == 0 and p + 1 <= (KT - 1) // 2:
                fill_window(h, p + 1)
            Qx = QaugA if p % 2 == 0 else QaugB
            kc = slice(h * S + kt * 128, h * S + (kt + 1) * 128)
            for qt, (q0, qw) in enumerate(QT):
                if queue and step_no % stride == 0:
                    queue.pop(0)()
                step_no += 1
                psS = ps.tile([128, 512], F32, name="psS", tag="s", bufs=3)
                nc.tensor.matmul(
                    psS[:, :qw], Kaug[0:120, kc],
                    Qx[0:120, h * S + q0: h * S + q0 + qw],
                    start=True, stop=True)
                pT = sb.tile([128, 512], BF16, name="pT", tag="p", bufs=4)
                nc.scalar.activation(out=pT[:, :qw], in_=psS[:, :qw],
                                     func=ACTF.Exp)
                trail.append((kt, qt, pT))
                if len(trail) >= 4:
                    pv_step(h, psOs, trail.pop(0))
        for step in trail:
            pv_step(h, psOs, step)

        # normalize: outT = psO[0:64] * broadcast(1/l); the broadcast matmul
        # borrows a rotating "s" psum buffer. One-step software pipeline so
        # PE and DVE stream instead of ping-ponging.
        psBs = {}

        def norm_tail(qt):
            q0, qw = QT[qt]
            lrb = sb.tile([HD, 512], BF16, name="lrb", tag="lrb", bufs=2)
            nc.vector.tensor_copy(lrb[:, :qw], psBs[qt][0:HD, :qw])
            nc.vector.tensor_mul(T["outT"][h][:, q0:q0 + qw],
                                 psOs[qt][0:HD, :qw], lrb[:, :qw])

        prev = None
        for qt, (q0, qw) in enumerate(QT):
            lr = sb.tile([1, 512], BF16, name="lr", tag="lr", bufs=2)
            with nc.allow_low_precision(reason="1/l in bf16: uniform 2^-9 "
                                        "noise, gate is 2e-2"):
                nc.vector.reciprocal(out=lr[:, :qw],
                                     in_=psOs[qt][HD:HD + 1, :qw])
            psB = ps.tile([128, 512], F32, name="psB", tag="s", bufs=3)
            nc.tensor.matmul(psB[0:HD, :qw], C["ones64"], lr[0:1, :qw],
                             start=True, stop=True)
            psBs[qt] = psB
            if prev is not None:
                norm_tail(prev)
            prev = qt
        norm_tail(prev)

    # leftover interleave items (shouldn't normally remain)
    for it in queue:
        it()


def build_nc(num_devices=N_CORES, reps=1, diag=None):
    from contextlib import ExitStack
    nc = bacc.Bacc("TRN2", target_bir_lowering=False, debug=False,
                   num_devices=num_devices)
    aps = (
        nc.dram_tensor("xT", [D, S], BF16, kind="ExternalInput").ap(),
        nc.dram_tensor("wqk", [D, 2 * NHC * HD], BF16, kind="ExternalInput").ap(),
        nc.dram_tensor("bqk", [128, NHC], F32, kind="ExternalInput").ap(),
        nc.dram_tensor("wv", [D, NHC * HD], BF16, kind="ExternalInput").ap(),
        nc.dram_tensor("wp", [NHC, HD, D], BF16, kind="ExternalInput").ap(),
        nc.dram_tensor("RhT", [HD, S], BF16, kind="ExternalInput").ap(),
        nc.dram_tensor("RwT", [HD, S], BF16, kind="ExternalInput").ap(),
        nc.dram_tensor("Estat", [56, S], BF16, kind="ExternalInput").ap(),
        nc.dram_tensor("y", [S, D], F32, kind="ExternalOutput").ap(),
    )
    with tile.TileContext(nc) as tc:
        with ExitStack() as es:
            sb = es.enter_context(tc.tile_pool(name="sb", bufs=1))
            ps = es.enter_context(tc.tile_pool(name="ps", bufs=1,
                                               space="PSUM"))
            # software-pipelined across reps: rep r's attention emission
            # interleaves rep r+1's prologue and rep r-1's projection
            C = _alloc_const_tiles(sb)
            if diag in ("attn", "attn_nofill"):
                # diagnostic: attention stream only, operands memset once
                T0 = _alloc_rep_tiles(sb)
                nc.vector.memset(C["ones64"], 1.0)
                for t in (T0["Kaug"], T0["QaugA"], T0["QaugB"], T0["relh"],
                          T0["v"]):
                    nc.vector.memset(t, 0.01)
                for j in range(NHC):
                    nc.vector.memset(T0["outT"][j], 0.0)
                for r in range(reps):
                    _attention(nc, ps, sb, T0, C, [],
                               no_fills=(diag == "attn_nofill"))
                nc.gpsimd.dma_start(out=aps[8][0:64, :],
                                     in_=T0["outT"][0][0:64, 0:D])
            else:
                T_cur = _alloc_rep_tiles(sb)
                for it in _prologue_items(nc, aps, ps, T_cur, C, first=True):
                    it()
                proj_prev = []
                for r in range(reps):
                    nxt = []
                    T_nxt = None
                    if r + 1 < reps:
                        T_nxt = _alloc_rep_tiles(sb)
                        nxt = _prologue_items(nc, aps, ps, T_nxt, C,
                                              first=False)
                    _attention(nc, ps, sb, T_cur, C, proj_prev + nxt)
                    proj_prev = _proj_items(nc, aps, ps, sb, T_cur, C)
                    if r == reps - 1:
                        for it in proj_prev:
                            it()
                    T_cur = T_nxt
    nc.compile()
    return nc


def prep_core_inputs(c, x, qkv_w, qkv_b, proj_w, rel_pos_h, rel_pos_w):
    bf16 = mybir.dt.np(BF16)
    b = c // 4
    heads = [3 * (c % 4) + j for j in range(NHC)]
    f32 = np.float32
    xTa = np.ascontiguousarray(np.asarray(x, f32)[b].reshape(S, D).T).astype(bf16)
    qkv_w = np.asarray(qkv_w, f32)
    qkv_b = np.asarray(qkv_b, f32)
    wq = np.concatenate([qkv_w[:, h * HD:(h + 1) * HD] for h in heads], 1) * f32(SCALE)
    wk = np.concatenate([qkv_w[:, D + h * HD:D + (h + 1) * HD] for h in heads], 1)
    wqka = np.ascontiguousarray(np.concatenate([wq, wk], 1)).astype(bf16)
    bq = [qkv_b[h * HD:(h + 1) * HD] * f32(SCALE) for h in heads]
    bk = [qkv_b[D + h * HD:D + (h + 1) * HD] for h in heads]
    # per-M-tile half-stacked biases: [q0|q1], [q2|k0], [k1|k2]
    halves = [bq[0], bq[1], bq[2], bk[0], bk[1], bk[2]]
    bqka = np.stack([np.concatenate([halves[2 * m], halves[2 * m + 1]])
                     for m in range(NHC)], 1).astype(f32)
    wva = np.ascontiguousarray(
        np.concatenate([qkv_w[:, 2 * D + h * HD:2 * D + (h + 1) * HD]
                        for h in heads], 1)).astype(bf16)
    wpa = np.ascontiguousarray(
        np.stack([np.asarray(proj_w, f32)[h * HD:(h + 1) * HD, :]
                  for h in heads], 0)).astype(bf16)
    coords = np.arange(H)[:, None] - np.arange(H)[None, :] + (H - 1)
    Rh = np.asarray(rel_pos_h, f32)[coords]      # [hq, hk, c]
    Rw = np.asarray(rel_pos_w, f32)[coords]      # [wq, wk, c]
    # The reference builds the rel bias from the UNSCALED q; we fold `SCALE`
    # into wq/bq, so fold the exact inverse (8.0) into the rel tables.
    inv = f32(1.0 / SCALE)
    RhTa = (np.ascontiguousarray(np.transpose(Rh, (2, 0, 1)).reshape(HD, S))
            * inv).astype(bf16)
    RwTa = (np.ascontiguousarray(np.transpose(Rw, (2, 0, 1)).reshape(HD, S))
            * inv).astype(bf16)
    # static one-hots for the augmented-K logits matmul: rows 0-47 rel_w
    # (k%48), rows 48-55 rel_h window selector (k//48 - pair base)
    E = np.zeros((56, S), bf16)
    kk = np.arange(S)
    E[kk % W, kk] = 1.0
    jj = kk // W - (256 * (kk // 256)) // W
    E[48 + jj, kk] = 1.0
    return {"xT": xTa, "wqk": wqka, "bqk": bqka, "wv": wva, "wp": wpa,
            "RhT": RhTa, "RwT": RwTa, "Estat": E}


_NC_CACHE = {}


def _get_nc(**kw):
    key = str(sorted(kw.items()))
    if key not in _NC_CACHE:
        _NC_CACHE[key] = build_nc(**kw)
    return _NC_CACHE[key]


def gather_output(ys, qkv_b, proj_w, proj_b):
    f32 = np.float32
    bp_eff = (np.asarray(proj_b, f32)
              + np.asarray(qkv_b, f32)[2 * D:] @ np.asarray(proj_w, f32))
    out = np.empty((B, H, W, D), f32)
    for b in range(B):
        acc = ys[4 * b].copy()
        for j in range(1, 4):
            acc += ys[4 * b + j]
        acc += bp_eff
        out[b] = acc.reshape(H, W, D)
    return out


def kernel(x, qkv_w, qkv_b, proj_w, proj_b, rel_pos_h, rel_pos_w):
    from concourse.bass_utils import run_bass_kernel_spmd
    nc = _get_nc()
    in_maps = [prep_core_inputs(c, x, qkv_w, qkv_b, proj_w, rel_pos_h, rel_pos_w)
               for c in range(N_CORES)]
    res = run_bass_kernel_spmd(nc, in_maps, core_ids=list(range(N_CORES)))
    ys = [res.results[c]["y"] for c in range(N_CORES)]
    return gather_output(ys, qkv_b, proj_w, proj_b)


# revision 36
# speedup vs baseline: 12311.8126x; 1.5397x over previous
"""Trainium2 Bass kernel for nn_Attention_15857019256917 (ViTDet-style attention
with decomposed relative position bias).

Sharding: data-parallel over B (2) x head-parallel (12 heads -> 4 groups of 3)
= 8 cores. Each core computes 3 heads of attention for one batch element plus
its partial output projection (rows of proj_w for its heads); the host sums the
4 partials per batch element (row-parallel linear unshard) and adds the bias
(with the v-bias folded in exactly: P@(V+1 bv^T)/l = PV/l + bv^T).

bf16 datapath (rel-err gate is 2e-2; measured ~5e-3): all matmul operands are
bf16 (1 PE cycle/col vs fp32's 4), PSUM accumulation stays fp32.

The logits for one k-tile take a single 120-row matmul: the stationary
operand stacks k (rows 0-63) with one-hot rows selecting rel_w (64-111,
k%48) and a windowed rel_h (112-119, k//48 - pair base); the moving operand
stacks q with the per-head rel_w / rel_h-window tables. Two Qaug variants
alternate per k-tile pair so the next pair's window DMA never touches rows
the in-flight matmuls read ([0:120] excludes the spare rows). The PV matmul
trails the logits by three steps so the softmax exp (ACT is the throughput
floor) always overlaps PE work.

Engine assignment: PE all matmuls (incl. the 1/l broadcast); ACT exp only;
DVE psum->sbuf copies, qkv bias adds, reciprocal, normalization multiply;
SWDGE (Pool-issued) side DMAs (rel tables, one-hots, rel_h windows); SP
HWDGE queue x/weight loads, parked-half shifts and y stores.

All tiles come from two pools that live across the whole NEFF (no per-phase
pool scopes, so no all-engine barriers between phases). In a reps>1 NEFF the
emission itself is software-pipelined across kernel instances: rep r's
ACT-bound attention stream interleaves rep r+1's prologue (input loads, qkv
projection, V, rel tables) and rep r-1's output projection as closures
popped between attention steps; tensors written by rep r+1's prologue while
rep r still reads them (Kaug/Qaug/relh/v/outT) are double-buffered. Engines
execute their queues in order, so this emission interleave is what converts
per-rep latency into pipelined throughput.
"""
import sys

sys.path.insert(0, "/opt/trn_rl_repo")

import numpy as np

import concourse.bass as bass
import concourse.bacc as bacc
import concourse.tile as tile
from concourse import mybir

F32 = mybir.dt.float32
BF16 = mybir.dt.bfloat16
ACTF = mybir.ActivationFunctionType

B, H, W, D = 2, 48, 48, 768
NH, HD = 12, 64
S = H * W                      # 2304
SCALE = HD ** -0.5
N_CORES = 8
NHC = 3                        # heads per core
KT = S // 128                  # 18 key tiles
TOKT = S // 128                # 18 token tiles
KCH = D // 128                 # 6 contraction chunks
QT = [(0, 512), (512, 512), (1024, 512), (1536, 512), (2048, 256)]
NQT = len(QT)
VST = NHC * (HD + 1)           # 195: per-ktile V layout [v_h0|1|v_h1|1|v_h2|1]


def _ap(t, off_elems, dims):
    """Raw AP on tile t: partition dim copied, free dims = [[step, count], ...]."""
    return bass.AP(tensor=t.tensor, offset=t.offset + off_elems, ap=[t.ap[0]] + dims)


def _alloc_rep_tiles(sb):
    """Per-rep rotating SBUF tiles. bufs=2 on everything rep r+1's prologue
    writes while rep r's attention still reads."""
    T = {}
    T["Kaug"] = sb.tile([128, NHC * S], BF16, name="Kaug", tag="Kaug", bufs=2)
    T["QaugA"] = sb.tile([128, NHC * S], BF16, name="QaugA", tag="QaugA", bufs=2)
    T["QaugB"] = sb.tile([128, NHC * S], BF16, name="QaugB", tag="QaugB", bufs=2)
    T["relh"] = sb.tile([48, NHC * S], BF16, name="relh", tag="relh", bufs=2)
    T["outT"] = [sb.tile([HD, S], BF16, name=f"outT{j}", tag=f"outT{j}",
                         bufs=2)
                 for j in range(NHC)]
    T["v"] = sb.tile([128, TOKT * VST], BF16, name="v", tag="v", bufs=2)
    T["xs"] = [sb.tile([128, S], BF16, name=f"x{k}", tag=f"x{k}")
               for k in range(KCH)]
    T["wqk_sb"] = sb.tile([128, KCH * 2 * NHC * HD], BF16, name="wqk_sb",
                          tag="wqk_sb")
    T["wv_sb"] = sb.tile([128, KCH * NHC * HD], BF16, name="wv_sb",
                         tag="wv_sb")
    T["bqk_sb"] = sb.tile([128, NHC], F32, name="bqk_sb", tag="bqk_sb")
    return T


def _alloc_const_tiles(sb):
    """Constants shared by every rep: one buffer, loaded once."""
    C = {}
    C["RhT_sb"] = sb.tile([HD, S], BF16, name="RhT_sb", tag="RhT_sb")
    C["RwT_sb"] = sb.tile([HD, S], BF16, name="RwT_sb", tag="RwT_sb")
    C["wp_sb"] = [sb.tile([HD, D], BF16, name=f"wp{j}", tag=f"wp{j}")
                  for j in range(NHC)]
    C["ones64"] = sb.tile([1, HD], BF16, name="ones64", tag="ones64")
    return C


def _prologue_items(nc, aps, ps, T, C, first):
    """Emission closures for one rep's prologue (input loads, qkv
    projections, V, rel tables), to be interleaved into the previous rep's
    ACT-bound attention stream so this PE/DMA work hides under it."""
    xT, wqk, bqk, wv, wp, RhT, RwT, Estat, y = aps
    Kaug, QaugA, QaugB = T["Kaug"], T["QaugA"], T["QaugB"]
    relh, v, xs = T["relh"], T["v"], T["xs"]
    wqk_sb, wv_sb, bqk_sb = T["wqk_sb"], T["wv_sb"], T["bqk_sb"]
    items = []

    def loads():
        if first:
            # constants shared by every rep: load once (also avoids a
            # cross-rep WAR between these loads and the previous rep's
            # interleaved projection reads)
            nc.gpsimd.dma_start(out=C["RhT_sb"], in_=RhT)
            nc.gpsimd.dma_start(out=C["RwT_sb"], in_=RwT)
            for j in range(NHC):
                nc.gpsimd.dma_start(out=C["wp_sb"][j], in_=wp[j])
            nc.vector.memset(C["ones64"], 1.0)
        nc.sync.dma_start(out=bqk_sb, in_=bqk)
        nc.vector.memset(_ap(v, HD, [[VST, TOKT], [HD + 1, NHC]]), 1.0)
        for k in range(KCH):
            nc.sync.dma_start(out=xs[k], in_=xT[k * 128:(k + 1) * 128, :])
        for k in range(KCH):
            nc.sync.dma_start(out=wqk_sb[:, k * 384:(k + 1) * 384],
                              in_=wqk[k * 128:(k + 1) * 128, :])
            nc.sync.dma_start(out=wv_sb[:, k * 192:(k + 1) * 192],
                              in_=wv[k * 128:(k + 1) * 128, :])
        # static one-hot rows of Kaug (high halves park in QaugB, so no
        # ordering dependency on phase 1)
        for j in range(NHC):
            nc.gpsimd.dma_start(out=Kaug[64:112, j * S:(j + 1) * S],
                                in_=Estat[0:48, :])
            nc.gpsimd.dma_start(out=Kaug[112:120, j * S:(j + 1) * S],
                                in_=Estat[48:56, :])
    items.append(loads)

    # M-tiles (128 rows = two 64-channel halves): T0=[q0|q1] T1=[q2|k0]
    # T2=[k1|k2]. Low halves copy straight to rows 0-63 of their dest
    # tensor; high halves park in QaugB rows 64-127 (same column range),
    # then an intra-tensor DMA partition-shifts them to the dest.
    lo_dest = [(QaugA, 0), (QaugA, 2), (Kaug, 1)]
    hi_dest = [(QaugA, 1), (Kaug, 0), (Kaug, 2)]
    for m in range(NHC):
        for (n0, nw) in QT:
            def qk_group(m=m, n0=n0, nw=nw):
                psq = ps.tile([128, 512], F32, name="psq", tag="s", bufs=3)
                for k in range(KCH):
                    nc.tensor.matmul(
                        psq[:, :nw],
                        wqk_sb[:, k * 384 + m * 128: k * 384 + (m + 1) * 128],
                        xs[k][:, n0:n0 + nw],
                        start=(k == 0), stop=(k == KCH - 1))
                lt_, lh = lo_dest[m]
                ht_, hh = hi_dest[m]
                nc.vector.tensor_scalar_add(
                    lt_[0:64, lh * S + n0: lh * S + n0 + nw],
                    psq[0:64, :nw], bqk_sb[0:64, m:m + 1])
                nc.vector.tensor_scalar_add(
                    QaugB[64:128, hh * S + n0: hh * S + n0 + nw],
                    psq[64:128, :nw], bqk_sb[64:128, m:m + 1])
            items.append(qk_group)
    for ts in range(TOKT):
        def v_group(ts=ts):
            psv = ps.tile([128, 512], F32, name="psv", tag="s", bufs=3)
            for k in range(KCH):
                nc.tensor.matmul(
                    psv[:, 0:NHC * HD],
                    xs[k][:, ts * 128:(ts + 1) * 128],
                    wv_sb[:, k * 192:(k + 1) * 192],
                    start=(k == 0), stop=(k == KCH - 1))
            vdst = _ap(v, ts * VST, [[HD + 1, NHC], [1, HD]])
            vsrc = _ap(psv, 0, [[HD, NHC], [1, HD]])
            nc.vector.tensor_copy(vdst, vsrc)
        items.append(v_group)

    def shifts():
        for m in range(NHC):
            ht_, hh = hi_dest[m]
            nc.sync.dma_start(out=ht_[0:64, hh * S:(hh + 1) * S],
                              in_=QaugB[64:128, hh * S:(hh + 1) * S])
        # q lives in both Qaug variants (Pool: SBUF->SBUF, keeps DVE free)
        nc.gpsimd.tensor_copy(QaugB[0:64, :], QaugA[0:64, :])
    items.append(shifts)

    # rel tables (PE-only): rel_h lands at psum rows 0-47 -> relh staging
    # tile; rel_w at rows 64-111 -> both Qaug variants
    for h in range(NHC):
        for g in range(5):
            def rel_group(h=h, g=g):
                cnt = 10 if g < 4 else 8
                psr = ps.tile([128, 512], F32, name="psr", tag="s", bufs=3)
                for i in range(cnt):
                    r = g * 10 + i
                    nc.tensor.matmul(
                        psr[0:48, i * 48:(i + 1) * 48],
                        C["RhT_sb"][:, r * 48:(r + 1) * 48],
                        QaugA[0:64, h * S + r * 48: h * S + (r + 1) * 48],
                        start=(i == 0), stop=(i == cnt - 1))
                    # out at base partition 64 (col-tiled); the sim's
                    # zero-region bookkeeping mis-indexes partition-offset
                    # psum APs, so skip its group check (single writer)
                    nc.tensor.matmul(
                        psr[64:112, i * 48:(i + 1) * 48],
                        C["RwT_sb"][:, r * 48:(r + 1) * 48],
                        bass.AP(tensor=QaugA.tensor,
                                offset=QaugA.offset + h * S + r,
                                ap=[QaugA[0:64, :].ap[0], [48, 48]]),
                        start=(i == 0), stop=(i == cnt - 1),
                        skip_group_check=True)
                nc.vector.tensor_copy(
                    relh[0:48, h * S + g * 480: h * S + g * 480 + cnt * 48],
                    psr[0:48, 0:cnt * 48])
                wsrc = bass.AP(tensor=psr.tensor,
                               offset=psr[64:112, :].offset,
                               ap=[psr[64:112, :].ap[0], [48, cnt], [1, 48]])
                for Qx in (QaugA, QaugB):
                    wdst = bass.AP(
                        tensor=Qx.tensor,
                        offset=Qx[64:112, :].offset + h * S + g * 10,
                        ap=[Qx[64:112, :].ap[0], [1, cnt], [48, 48]])
                    nc.vector.tensor_copy(wdst, wsrc)
            items.append(rel_group)
    return items


def _proj_items(nc, aps, ps, sb, T, C):
    """Output-projection closures for a finished rep: interleaved into the
    next rep's attention (or emitted directly for the last rep). Split per
    psum tile so at most one extra "s" buffer is held at a time."""
    y = aps[8]
    items = []
    for ts in range(TOKT):
        def proj_a(ts=ts):
            psA = ps.tile([128, 512], F32, name="psA", tag="s", bufs=3)
            for j in range(NHC):
                nc.tensor.matmul(psA,
                                 T["outT"][j][:, ts * 128:(ts + 1) * 128],
                                 C["wp_sb"][j][:, 0:512],
                                 start=(j == 0), stop=(j == NHC - 1))
            y_sb = sb.tile([128, D], F32, name="y_sb", tag="ysb", bufs=1)
            nc.vector.tensor_copy(y_sb[:, 0:512], psA)
            T["_ysb"] = y_sb
        def proj_b(ts=ts):
            psB2 = ps.tile([128, 512], F32, name="psB2", tag="s", bufs=3)
            for j in range(NHC):
                nc.tensor.matmul(psB2[:, 0:256],
                                 T["outT"][j][:, ts * 128:(ts + 1) * 128],
                                 C["wp_sb"][j][:, 512:768],
                                 start=(j == 0), stop=(j == NHC - 1))
            y_sb = T["_ysb"]
            nc.vector.tensor_copy(y_sb[:, 512:768], psB2[:, 0:256])
            nc.sync.dma_start(out=y[ts * 128:(ts + 1) * 128, :], in_=y_sb)
        items.append(proj_a)
        items.append(proj_b)
    return items


def _attention(nc, ps, sb, T, C, interleave, no_fills=False):
    """k-tile-outer attention for all heads. One 120-row matmul per
    (kt, qt) yields QK^T + both rel biases; PV trails the logits by two
    steps so the exp always overlaps PE work. `interleave` closures (next
    rep's prologue, previous rep's projection) are popped between steps to
    fill PE's idle time under the ACT-bound exp stream."""
    Kaug, QaugA, QaugB, relh, v = (T["Kaug"], T["QaugA"], T["QaugB"],
                                   T["relh"], T["v"])
    queue = list(interleave)
    nsteps = NHC * KT * NQT
    stride = max(1, nsteps // (len(queue) + 1)) if queue else nsteps
    step_no = 0

    def fill_window(h, p):
        if no_fills:
            return
        Qx = QaugA if p % 2 == 0 else QaugB
        base = (256 * p) // 48
        n = min(8, 48 - base)
        nc.gpsimd.dma_start(out=Qx[112:112 + n, h * S:(h + 1) * S],
                            in_=relh[base:base + n, h * S:(h + 1) * S])

    def pv_step(h, psOs, step):
        kt, qt, pT = step
        q0, qw = QT[qt]
        vsl = slice(kt * VST + h * (HD + 1), kt * VST + (h + 1) * (HD + 1))
        nc.tensor.matmul(psOs[qt][:, :qw], v[:, vsl], pT[:, :qw],
                         start=(kt == 0), stop=(kt == KT - 1))

    for h in range(NHC):
        psOs = [ps.tile([HD + 1, 512], F32, name=f"psO{h}{qt}", tag=f"o{qt}")
                for qt in range(NQT)]
        fill_window(h, 0)
        fill_window(h, 1)
        trail = []
        for kt in range(KT):
            p = kt // 2
            if kt >= 2 and kt % 2 == 0 and p + 1 <= (KT - 1) // 2:
                fill_window(h, p + 1)
            Qx = QaugA if p % 2 == 0 else QaugB
            kc = slice(h * S + kt * 128, h * S + (kt + 1) * 128)
            for qt, (q0, qw) in enumerate(QT):
                if queue and step_no % stride == 0:
                    queue.pop(0)()
                step_no += 1
                psS = ps.tile([128, 512], F32, name="psS", tag="s", bufs=3)
                nc.tensor.matmul(
                    psS[:, :qw], Kaug[0:120, kc],
                    Qx[0:120, h * S + q0: h * S + q0 + qw],
                    start=True, stop=True)
                pT = sb.tile([128, 512], BF16, name="pT", tag="p", bufs=4)
                nc.scalar.activation(out=pT[:, :qw], in_=psS[:, :qw],
                                     func=ACTF.Exp)
                trail.append((kt, qt, pT))
                if len(trail) >= 4:
                    pv_step(h, psOs, trail.pop(0))
        for step in trail:
            pv_step(h, psOs, step)

        # normalize: outT = psO[0:64] * broadcast(1/l); the broadcast matmul
        # borrows a rotating "s" psum buffer. One-step software pipeline so
        # PE and DVE stream instead of ping-ponging.
        psBs = {}

        def norm_tail(qt):
            q0, qw = QT[qt]
            lrb = sb.tile([HD, 512], BF16, name="lrb", tag="lrb", bufs=2)
            nc.vector.tensor_copy(lrb[:, :qw], psBs[qt][0:HD, :qw])
            nc.vector.tensor_mul(T["outT"][h][:, q0:q0 + qw],
                                 psOs[qt][0:HD, :qw], lrb[:, :qw])

        prev = None
        for qt, (q0, qw) in enumerate(QT):
            lr = sb.tile([1, 512], BF16, name="lr", tag="lr", bufs=2)
            with nc.allow_low_precision(reason="1/l in bf16: uniform 2^-9 "
                                        "noise, gate is 2e-2"):
                nc.vector.reciprocal(out=lr[:, :qw],
                                     in_=psOs[qt][HD:HD + 1, :qw])
            psB = ps.tile([128, 512], F32, name="psB", tag="s", bufs=3)
            nc.tensor.matmul(psB[0:HD, :qw], C["ones64"], lr[0:1, :qw],
                             start=True, stop=True)
            psBs[qt] = psB
            if prev is not None:
                norm_tail(prev)
            prev = qt
        norm_tail(prev)

    # leftover interleave items (shouldn't normally remain)
    for it in queue:
        it()


def build_nc(num_devices=N_CORES, reps=1, diag=None):
    from contextlib import ExitStack
    nc = bacc.Bacc("TRN2", target_bir_lowering=False, debug=False,
                   num_devices=num_devices)
    aps = (
        nc.dram_tensor("xT", [D, S], BF16, kind="ExternalInput").ap(),
        nc.dram_tensor("wqk", [D, 2 * NHC * HD], BF16, kind="ExternalInput").ap(),
        nc.dram_tensor("bqk", [128, NHC], F32, kind="ExternalInput").ap(),
        nc.dram_tensor("wv", [D, NHC * HD], BF16, kind="ExternalInput").ap(),
        nc.dram_tensor("wp", [NHC, HD, D], BF16, kind="ExternalInput").ap(),
        nc.dram_tensor("RhT", [HD, S], BF16, kind="ExternalInput").ap(),
        nc.dram_tensor("RwT", [HD, S], BF16, kind="ExternalInput").ap(),
        nc.dram_tensor("Estat", [56, S], BF16, kind="ExternalInput").ap(),
        nc.dram_tensor("y", [S, D], F32, kind="ExternalOutput").ap(),
    )
    with tile.TileContext(nc) as tc:
        with ExitStack() as es:
            sb = es.enter_context(tc.tile_pool(name="sb", bufs=1))
            ps = es.enter_context(tc.tile_pool(name="ps", bufs=1,
                                               space="PSUM"))
            # software-pipelined across reps: rep r's attention emission
            # interleaves rep r+1's prologue and rep r-1's projection
            C = _alloc_const_tiles(sb)
            if diag in ("attn", "attn_nofill"):
                # diagnostic: attention stream only, operands memset once
                T0 = _alloc_rep_tiles(sb)
                nc.vector.memset(C["ones64"], 1.0)
                for t in (T0["Kaug"], T0["QaugA"], T0["QaugB"], T0["relh"],
                          T0["v"]):
                    nc.vector.memset(t, 0.01)
                for j in range(NHC):
                    nc.vector.memset(T0["outT"][j], 0.0)
                for r in range(reps):
                    _attention(nc, ps, sb, T0, C, [],
                               no_fills=(diag == "attn_nofill"))
                nc.gpsimd.dma_start(out=aps[8][0:64, :],
                                     in_=T0["outT"][0][0:64, 0:D])
            else:
                T_cur = _alloc_rep_tiles(sb)
                for it in _prologue_items(nc, aps, ps, T_cur, C, first=True):
                    it()
                proj_prev = []
                for r in range(reps):
                    nxt = []
                    T_nxt = None
                    if r + 1 < reps:
                        T_nxt = _alloc_rep_tiles(sb)
                        nxt = _prologue_items(nc, aps, ps, T_nxt, C,
                                              first=False)
                    _attention(nc, ps, sb, T_cur, C, proj_prev + nxt)
                    proj_prev = _proj_items(nc, aps, ps, sb, T_cur, C)
                    if r == reps - 1:
                        for it in proj_prev:
                            it()
                    T_cur = T_nxt
    nc.compile()
    return nc


def prep_core_inputs(c, x, qkv_w, qkv_b, proj_w, rel_pos_h, rel_pos_w):
    bf16 = mybir.dt.np(BF16)
    b = c // 4
    heads = [3 * (c % 4) + j for j in range(NHC)]
    f32 = np.float32
    xTa = np.ascontiguousarray(np.asarray(x, f32)[b].reshape(S, D).T).astype(bf16)
    qkv_w = np.asarray(qkv_w, f32)
    qkv_b = np.asarray(qkv_b, f32)
    wq = np.concatenate([qkv_w[:, h * HD:(h + 1) * HD] for h in heads], 1) * f32(SCALE)
    wk = np.concatenate([qkv_w[:, D + h * HD:D + (h + 1) * HD] for h in heads], 1)
    wqka = np.ascontiguousarray(np.concatenate([wq, wk], 1)).astype(bf16)
    bq = [qkv_b[h * HD:(h + 1) * HD] * f32(SCALE) for h in heads]
    bk = [qkv_b[D + h * HD:D + (h + 1) * HD] for h in heads]
    # per-M-tile half-stacked biases: [q0|q1], [q2|k0], [k1|k2]
    halves = [bq[0], bq[1], bq[2], bk[0], bk[1], bk[2]]
    bqka = np.stack([np.concatenate([halves[2 * m], halves[2 * m + 1]])
                     for m in range(NHC)], 1).astype(f32)
    wva = np.ascontiguousarray(
        np.concatenate([qkv_w[:, 2 * D + h * HD:2 * D + (h + 1) * HD]
                        for h in heads], 1)).astype(bf16)
    wpa = np.ascontiguousarray(
        np.stack([np.asarray(proj_w, f32)[h * HD:(h + 1) * HD, :]
                  for h in heads], 0)).astype(bf16)
    coords = np.arange(H)[:, None] - np.arange(H)[None, :] + (H - 1)
    Rh = np.asarray(rel_pos_h, f32)[coords]      # [hq, hk, c]
    Rw = np.asarray(rel_pos_w, f32)[coords]      # [wq, wk, c]
    # The reference builds the rel bias from the UNSCALED q; we fold `SCALE`
    # into wq/bq, so fold the exact inverse (8.0) into the rel tables.
    inv = f32(1.0 / SCALE)
    RhTa = (np.ascontiguousarray(np.transpose(Rh, (2, 0, 1)).reshape(HD, S))
            * inv).astype(bf16)
    RwTa = (np.ascontiguousarray(np.transpose(Rw, (2, 0, 1)).reshape(HD, S))
            * inv).astype(bf16)
    # static one-hots for the augmented-K logits matmul: rows 0-47 rel_w
    # (k%48), rows 48-55 rel_h window selector (k//48 - pair base)
    E = np.zeros((56, S), bf16)
    kk = np.arange(S)
    E[kk % W, kk] = 1.0
    jj = kk // W - (256 * (kk // 256)) // W
    E[48 + jj, kk] = 1.0
    return {"xT": xTa, "wqk": wqka, "bqk": bqka, "wv": wva, "wp": wpa,
            "RhT": RhTa, "RwT": RwTa, "Estat": E}


_NC_CACHE = {}


def _get_nc(**kw):
    key = str(sorted(kw.items()))
    if key not in _NC_CACHE:
        _NC_CACHE[key] = build_nc(**kw)
    return _NC_CACHE[key]


def gather_output(ys, qkv_b, proj_w, proj_b):
    f32 = np.float32
    bp_eff = (np.asarray(proj_b, f32)
              + np.asarray(qkv_b, f32)[2 * D:] @ np.asarray(proj_w, f32))
    out = np.empty((B, H, W, D), f32)
    for b in range(B):
        acc = ys[4 * b].copy()
        for j in range(1, 4):
            acc += ys[4 * b + j]
        acc += bp_eff
        out[b] = acc.reshape(H, W, D)
    return out


def kernel(x, qkv_w, qkv_b, proj_w, proj_b, rel_pos_h, rel_pos_w):
    from concourse.bass_utils import run_bass_kernel_spmd
    nc = _get_nc()
    in_maps = [prep_core_inputs(c, x, qkv_w, qkv_b, proj_w, rel_pos_h, rel_pos_w)
               for c in range(N_CORES)]
    res = run_bass_kernel_spmd(nc, in_maps, core_ids=list(range(N_CORES)))
    ys = [res.results[c]["y"] for c in range(N_CORES)]
    return gather_output(ys, qkv_b, proj_w, proj_b)
